# revision 22
# baseline (speedup 1.0000x reference)
import contextlib
import math
import os

import numpy as np

import concourse.bass as bass
import concourse.mybir as mybir
import concourse.tile as tile
from concourse import bacc
from concourse.bass_utils import run_bass_kernel_spmd
from concourse.masks import make_identity

# Model dims (hardcoded for nn_MoETransformerClassifier_76433237999751)
B, S, D, NH, HFF, E, TOPK, C, L, V = 8, 256, 1024, 16, 2048, 8, 2, 10, 4, 32000
HD = D // NH          # 64
T = S                 # 256 local tokens per core
KT = D // 128         # 8 k-tiles over D
MTH = HFF // 128      # 16 m-tiles over HFF
N_CORES = 8
EPS = 1e-5
TCH = 512             # token chunk for expert FFN
NCH = (N_CORES * T) // TCH  # 4 chunks of global tokens
RPC = TCH // T        # ranks per chunk = 2

F32 = mybir.dt.float32
F32R = mybir.dt.float32r
AF = mybir.ActivationFunctionType
ALU = mybir.AluOpType

# Per-matmul-group precision: "f32r" (full PE rate, rel err ~1.5e-4) or "f32"
# (exact, 4 cyc/row). Gate/stat/bias/select matmuls are always fp32.
# qkv/v read the fp32 residual stream directly -> fp32 (mixed dtypes illegal).
# av uses tile_position col-tiling, which fp32r's weight path cannot encode.
PREC = dict(qkv="f32", v="f32", score="f32r", av="f32", oproj="f32r",
            w1="f32r", w2="f32r")
_env = os.environ.get("KPREC")
if _env:
    for kv in _env.split(","):
        k, vv = kv.split("=")
        if k == "all":
            for kk in PREC:
                PREC[kk] = vv
        else:
            PREC[k] = vv


def _dt(group):
    return F32R if PREC[group] == "f32r" else F32


def _pos_encoding_np():
    pos = np.arange(5000, dtype=np.float32)[:, None]
    div = np.exp(np.arange(0, D, 2, dtype=np.float32) * (-math.log(10000.0) / D))
    pe = np.zeros((5000, D), dtype=np.float32)
    pe[:, 0::2] = np.sin(pos * div)
    pe[:, 1::2] = np.cos(pos * div)
    return pe[:B]  # [B, D]


def build_kernel():
    nc = bacc.Bacc(None, target_bir_lowering=False, debug=False,
                   num_devices=N_CORES)

    def din(name, shape, dtype=F32):
        return nc.dram_tensor(name, shape, dtype, kind="ExternalInput")

    emb_d = din("emb", [V, D])
    srcl_d = din("srcl", [2, 128], mybir.dt.int32)
    pe_d = din("pe", [KT, 128])
    onehot_d = din("onehot", [8, 1])
    wqkT_d = din("wqkT", [L, D, 2 * D], _dt("qkv"))
    bqk_d = din("bqk", [L, 16, 128])
    wvT_d = din("wvT", [L, D, D], _dt("v"))
    bv_d = din("bv", [L, 1, D])
    owT_d = din("owT", [L, D, D], _dt("oproj"))
    ob_d = din("ob", [L, 1, D])
    ln1g_d = din("ln1g", [L, KT, 128])
    ln1b_d = din("ln1b", [L, KT, 128])
    ln2g_d = din("ln2g", [L, KT, 128])
    ln2b_d = din("ln2b", [L, KT, 128])
    gwT_d = din("gwT", [L, D, E])
    gb_d = din("gb", [L, 1, E])
    w1e_d = din("w1e", [L, D, HFF], _dt("w1"))
    b1e_d = din("b1e", [L, 1, HFF])
    w2e_d = din("w2e", [L, HFF, D], _dt("w2"))
    b2e_d = din("b2e", [L, 1, D])
    selm_d = din("selm", [8, 8 * 128])
    clsT_d = din("clsT", [D, C])
    clsb_d = din("clsb", [1, C])

    cls_out_d = nc.dram_tensor("cls_out", [C, 1], F32, kind="ExternalOutput")
    gates_out_d = nc.dram_tensor("gates_out", [L, 2, 128, E], F32,
                                 kind="ExternalOutput")

    rg = [list(range(N_CORES))]

    with tile.TileContext(nc) as tc, contextlib.ExitStack() as ctx:
        dram = ctx.enter_context(tc.tile_pool(name="dram", bufs=1, space="DRAM"))
        singles = ctx.enter_context(tc.tile_pool(name="singles", bufs=1))
        p_x = ctx.enter_context(tc.tile_pool(name="p_x", bufs=2))
        p_big = ctx.enter_context(tc.tile_pool(name="p_big", bufs=1))
        p_xw = ctx.enter_context(tc.tile_pool(name="p_xw", bufs=1))
        p_oT = ctx.enter_context(tc.tile_pool(name="p_oT", bufs=1))
        p_w2 = ctx.enter_context(tc.tile_pool(name="p_w2", bufs=1))
        p_wst = ctx.enter_context(tc.tile_pool(name="p_wst", bufs=2))
        p_attn = ctx.enter_context(tc.tile_pool(name="p_attn", bufs=2))
        p_gth = ctx.enter_context(tc.tile_pool(name="p_gth", bufs=2))
        p_sm = ctx.enter_context(tc.tile_pool(name="p_sm", bufs=2))
        p_row = ctx.enter_context(tc.tile_pool(name="p_row", bufs=1))
        ppMM = ctx.enter_context(tc.tile_pool(name="ppMM", bufs=3, space="PSUM"))
        ppX = ctx.enter_context(tc.tile_pool(name="ppX", bufs=3, space="PSUM"))
        ppS = ctx.enter_context(tc.tile_pool(name="ppS", bufs=2, space="PSUM"))

        ident = singles.tile([128, 128], F32)
        make_identity(nc, ident[:])
        identr = singles.tile([128, 128], F32R)
        nc.vector.tensor_copy(identr, ident)
        ones_col = singles.tile([128, 1], F32)
        nc.vector.memset(ones_col, 1.0)
        ones_row = singles.tile([1, TCH], F32)
        nc.vector.memset(ones_row, 1.0)
        pe_s = singles.tile([128, KT], F32)
        nc.sync.dma_start(pe_s, pe_d.ap().rearrange("k p -> p k"))
        oh_s = singles.tile([8, 1], F32)
        nc.sync.dma_start(oh_s, onehot_d.ap())
        clsb_s = singles.tile([1, C], F32)
        nc.sync.dma_start(clsb_s, clsb_d.ap())
        one1 = singles.tile([1, 1], F32)
        nc.vector.memset(one1, 1.0)
        selm_s = singles.tile([8, 8 * 128], F32)
        nc.sync.dma_start(selm_s, selm_d.ap())
        eps_s = singles.tile([1, 1], F32)
        nc.vector.memset(eps_s, EPS)

        ag_in = dram.tile([KT * 128 + 8, T], F32)
        ag_out = dram.tile([N_CORES, KT * 128 + 8, T], F32)
        z_dram = dram.tile([N_CORES, KT, 128, T], F32)
        rs_out = dram.tile([KT, 128, T], F32)

        # ---------- embedding gather -> xT (feature-major) ----------
        xT = p_x.tile([128, KT, T], F32, tag="xbuf")
        for h in range(2):
            idx = p_sm.tile([128, 1], mybir.dt.int32, tag="idx")
            nc.sync.dma_start(idx, srcl_d.ap()[h].rearrange("(t o) -> t o", o=1))
            gt = p_xw.tile([128, D], F32, tag="vxw")
            nc.gpsimd.indirect_dma_start(
                out=gt[:], out_offset=None, in_=emb_d.ap(),
                in_offset=bass.IndirectOffsetOnAxis(ap=idx[:, :1], axis=0))
            for kt in range(KT):
                pt = ppX.tile([128, TCH], F32, tag="x")
                nc.tensor.transpose(pt[:, :128], gt[:, kt * 128:(kt + 1) * 128],
                                    ident[:])
                nc.scalar.activation(
                    out=xT[:, kt, h * 128:(h + 1) * 128], in_=pt[:, :128],
                    func=AF.Identity, scale=32.0, bias=pe_s[:, kt:kt + 1])

        def layernorm(xin, g_d, b_d, l):
            ps_sum = ppS.tile([1, T], F32, tag="sm")
            ps_sq = ppS.tile([1, T], F32, tag="sm")
            for kt in range(KT):
                sq_t = p_attn.tile([128, T], F32, tag="sqt")
                nc.scalar.activation(out=sq_t, in_=xin[:, kt], func=AF.Square)
                nc.tensor.matmul(ps_sum, ones_col[:], xin[:, kt],
                                 start=(kt == 0), stop=(kt == KT - 1))
                nc.tensor.matmul(ps_sq, ones_col[:], sq_t,
                                 start=(kt == 0), stop=(kt == KT - 1))
            mu = p_row.tile([1, T], F32, tag="rA")
            nc.scalar.activation(out=mu, in_=ps_sum, func=AF.Copy, scale=1.0 / D)
            ps_mu = ppX.tile([128, TCH], F32, tag="x")
            nc.tensor.matmul(ps_mu[:, :T], ones_row[:1, :128], mu,
                             start=True, stop=True)
            msq = p_row.tile([1, T], F32, tag="rB")
            nc.scalar.activation(out=msq, in_=ps_sq, func=AF.Copy, scale=1.0 / D)
            mu2 = p_row.tile([1, T], F32, tag="rC")
            nc.vector.tensor_mul(mu2, mu, mu)
            var = p_row.tile([1, T], F32, tag="rA")
            nc.vector.tensor_tensor(var, msq, mu2, ALU.subtract)
            vpe = p_row.tile([1, T], F32, tag="rB")
            nc.vector.tensor_scalar_add(vpe, var, EPS)
            lnv = p_row.tile([1, T], F32, tag="rC")
            nc.scalar.activation(out=lnv, in_=var, func=AF.Ln, bias=eps_s[:1])
            r0 = p_row.tile([1, T], F32, tag="rA")
            nc.scalar.activation(out=r0, in_=lnv, func=AF.Exp, scale=-0.5)
            # one Newton step: r' = r0*(1.5 - 0.5*(var+eps)*r0^2)
            rr = p_row.tile([1, T], F32, tag="rC")
            nc.vector.tensor_mul(rr, r0, r0)
            hv = p_row.tile([1, T], F32, tag="rD")
            nc.vector.tensor_scalar_mul(hv, vpe, -0.5)
            a1 = p_row.tile([1, T], F32, tag="rB")
            nc.vector.tensor_mul(a1, hv, rr)
            nc.vector.tensor_scalar_add(a1, a1, 1.5)
            rstd = p_row.tile([1, T], F32, tag="rC")
            nc.vector.tensor_mul(rstd, r0, a1)
            ps_rs = ppX.tile([128, TCH], F32, tag="x")
            nc.tensor.matmul(ps_rs[:, :T], ones_row[:1, :128], rstd,
                             start=True, stop=True)
            g_s = p_sm.tile([128, KT], F32, tag="lng")
            nc.sync.dma_start(g_s, g_d.ap()[l].rearrange("k p -> p k"))
            b_s = p_sm.tile([128, KT], F32, tag="lnb")
            nc.sync.dma_start(b_s, b_d.ap()[l].rearrange("k p -> p k"))
            xout = p_x.tile([128, KT, T], F32, tag="xbuf")
            for kt in range(KT):
                tt = p_attn.tile([128, T], F32, tag="lntmp")
                nc.vector.tensor_tensor(tt, xin[:, kt], ps_mu[:, :T], ALU.subtract)
                nc.vector.tensor_tensor(tt, tt, ps_rs[:, :T], ALU.mult)
                nc.scalar.activation(out=xout[:, kt], in_=tt, func=AF.Identity,
                                     scale=g_s[:, kt:kt + 1],
                                     bias=b_s[:, kt:kt + 1])
            return xout

        x_cur = xT
        for l in range(L):
            # ---------- attention: QK (feature-major) ----------
            qk_s = p_big.tile([128, 16, T], _dt("score"), tag="big")
            bqk_s = p_sm.tile([128, 16], F32, tag="bqk")
            nc.sync.dma_start(bqk_s, bqk_d.ap()[l].rearrange("m p -> p m"))
            for mt in range(16):
                wt = p_wst.tile([128, KT, 128], _dt("qkv"), tag="wtile")
                nc.sync.dma_start(
                    wt, wqkT_d.ap()[l].rearrange("(k p) m -> p k m", p=128)
                    [:, :, mt * 128:(mt + 1) * 128])
                ps = ppMM.tile([128, TCH], F32, tag="mm")
                for kt in range(KT):
                    nc.tensor.matmul(ps[:, :T], wt[:, kt],
                                     x_cur[:, kt],
                                     start=(kt == 0), stop=(kt == KT - 1))
                nc.scalar.activation(out=qk_s[:, mt], in_=ps[:, :T],
                                     func=AF.Identity,
                                     bias=bqk_s[:, mt:mt + 1])
            # ---------- V (token-major) ----------
            v_s = p_xw.tile([128, KT, TCH], _dt("av"), tag="vxw")
            bv_s = p_row.tile([1, D], F32, tag="biasrow")
            nc.sync.dma_start(bv_s, bv_d.ap()[l])
            for mt_tok in range(2):
                ps0 = ppMM.tile([128, TCH], F32, tag="mm")
                ps1 = ppMM.tile([128, TCH], F32, tag="mm")
                pss = [ps0, ps1]
                for kt in range(KT):
                    vw = p_wst.tile([128, D], _dt("v"), tag="vw")
                    nc.sync.dma_start(vw, wvT_d.ap()[l][kt * 128:(kt + 1) * 128, :])
                    for nh2 in range(2):
                        nc.tensor.matmul(
                            pss[nh2],
                            x_cur[:, kt, mt_tok * 128:(mt_tok + 1) * 128],
                            vw[:, nh2 * 512:(nh2 + 1) * 512],
                            start=(kt == 0), stop=False)
                for nh2 in range(2):
                    nc.tensor.matmul(pss[nh2], ones_row[:1, :128],
                                     bv_s[:, nh2 * 512:(nh2 + 1) * 512],
                                     start=False, stop=True)
                    nc.scalar.copy(v_s[:, mt_tok * 2 + nh2, :], pss[nh2])
            # v layout: v_s[:, mt_tok*2 + nh2, c] = V[token tile mt_tok, feature nh2*512+c]
            # ---------- attention heads ----------
            oT_s = p_oT.tile([128, 8, T], _dt("oproj"), tag="oT")
            for h in range(NH):
                mt_q = h // 2
                r0 = (h % 2) * 64
                attnT = p_attn.tile([128, 2, T], _dt("av"), tag="attnT")
                for qh in range(2):
                    ps_sc = ppMM.tile([128, TCH], F32, tag="mm")
                    nc.tensor.matmul(
                        ps_sc[:, :T],
                        qk_s[r0:r0 + 64, mt_q, qh * 128:(qh + 1) * 128],
                        qk_s[r0:r0 + 64, 8 + mt_q, :],
                        start=True, stop=True)
                    mx = p_sm.tile([128, 1], F32, tag="mx")
                    nc.vector.reduce_max(mx, ps_sc[:, :T], axis=mybir.AxisListType.X)
                    nmx = p_sm.tile([128, 1], F32, tag="nmx")
                    nc.vector.tensor_scalar_mul(nmx, mx, -0.125)
                    esum = p_sm.tile([128, 1], F32, tag="esum")
                    ae = p_attn.tile([128, T], F32, tag="ae")
                    nc.scalar.activation(out=ae, in_=ps_sc[:, :T], func=AF.Exp,
                                         scale=0.125, bias=nmx, accum_out=esum)
                    rs = p_sm.tile([128, 1], F32, tag="rsum")
                    nc.vector.reciprocal(rs, esum)
                    an = p_attn.tile([128, T], _dt("av"), tag="an")
                    nc.vector.tensor_scalar_mul(an, ae, rs)
                    for kh in range(2):
                        pt = ppX.tile([128, TCH], _dt("av"), tag="x")
                        nc.tensor.transpose(
                            pt[:, :128], an[:, kh * 128:(kh + 1) * 128],
                            identr[:] if _dt("av") == F32R else ident[:])
                        nc.scalar.copy(attnT[:, kh, qh * 128:(qh + 1) * 128], pt[:, :128])
                if h % 2 == 0:
                    ps_o = ppMM.tile([128, TCH], F32, tag="mm")
                for kt2 in range(2):
                    # v slice for head h, token-tile kt2:
                    # d-range h*64:(h+1)*64 lives in nh2 = h//8, col h%8*64...
                    nh2 = (h * 64) // 512
                    c0 = (h * 64) % 512
                    vsl = v_s[:, kt2 * 2 + nh2, c0:c0 + 64]
                    nc.tensor.matmul(
                        ps_o[r0:r0 + 64, :T], vsl,
                        attnT[:, kt2, :],
                        start=(kt2 == 0), stop=(kt2 == 1),
                        tile_position=(0, r0))
                if h % 2 == 1:
                    nc.scalar.copy(oT_s[:, h // 2, :], ps_o[:, :T])
            # ---------- out-proj + residual ----------
            x2 = p_x.tile([128, KT, T], F32, tag="xbuf")
            ob_s = p_row.tile([1, D], F32, tag="biasrow")
            nc.sync.dma_start(ob_s, ob_d.ap()[l])
            for mt in range(KT):
                wt = p_wst.tile([128, KT, 128], _dt("oproj"), tag="wtile")
                nc.sync.dma_start(
                    wt, owT_d.ap()[l].rearrange("(k p) m -> p k m", p=128)
                    [:, :, mt * 128:(mt + 1) * 128])
                ps = ppMM.tile([128, TCH], F32, tag="mm")
                for kt in range(KT):
                    nc.tensor.matmul(ps[:, :T], wt[:, kt],
                                     oT_s[:, kt],
                                     start=(kt == 0), stop=False)
                nc.tensor.matmul(ps[:, :T],
                                 ob_s[:, mt * 128:(mt + 1) * 128],
                                 ones_row[:1, :T], start=False, stop=True)
                nc.vector.scalar_tensor_tensor(
                    out=x2[:, mt], in0=ps[:, :T], scalar=1.0, in1=x_cur[:, mt],
                    op0=ALU.mult, op1=ALU.add)
            u = layernorm(x2, ln1g_d, ln1b_d, l)
            # ---------- gate + top-2 ----------
            gw_s = p_sm.tile([128, KT, E], F32, tag="gw")
            nc.sync.dma_start(gw_s,
                              gwT_d.ap()[l].rearrange("(k p) e -> p k e", p=128))
            gb_s = p_row.tile([1, E], F32, tag="gb")
            nc.sync.dma_start(gb_s, gb_d.ap()[l])
            W_s = p_sm.tile([128, 2, E], F32, tag="Ws")
            for mt_tok in range(2):
                ps_g = ppS.tile([128, E], F32, tag="sm")
                for kt in range(KT):
                    nc.tensor.matmul(
                        ps_g, u[:, kt, mt_tok * 128:(mt_tok + 1) * 128],
                        gw_s[:, kt], start=(kt == 0), stop=False)
                nc.tensor.matmul(ps_g, ones_row[:1, :128], gb_s,
                                 start=False, stop=True)
                logit = p_sm.tile([128, E], F32, tag="logit")
                nc.vector.tensor_copy(logit, ps_g)
                nc.sync.dma_start(gates_out_d.ap()[l, mt_tok], logit)
                m1 = p_sm.tile([128, 1], F32, tag="m1")
                nc.vector.reduce_max(m1, logit, axis=mybir.AxisListType.X)
                eq1 = p_sm.tile([128, E], F32, tag="eq1")
                nc.vector.tensor_scalar(out=eq1, in0=logit, scalar1=m1,
                                        scalar2=None, op0=ALU.is_equal)
                msk = p_sm.tile([128, E], F32, tag="msk")
                nc.vector.scalar_tensor_tensor(
                    out=msk, in0=eq1, scalar=-1e30, in1=logit,
                    op0=ALU.mult, op1=ALU.add)
                m2 = p_sm.tile([128, 1], F32, tag="m2")
                nc.vector.reduce_max(m2, msk, axis=mybir.AxisListType.X)
                eq2 = p_sm.tile([128, E], F32, tag="eq2")
                nc.vector.tensor_scalar(out=eq2, in0=msk, scalar1=m2,
                                        scalar2=None, op0=ALU.is_equal)
                nm1 = p_sm.tile([128, 1], F32, tag="nm1")
                nc.vector.tensor_scalar_mul(nm1, m1, -1.0)
                e2 = p_sm.tile([128, 1], F32, tag="e2")
                nc.scalar.activation(out=e2, in_=m2, func=AF.Exp, bias=nm1)
                den = p_sm.tile([128, 1], F32, tag="den")
                nc.vector.tensor_scalar_add(den, e2, 1.0)
                inv = p_sm.tile([128, 1], F32, tag="inv")
                nc.vector.reciprocal(inv, den)
                w2c = p_sm.tile([128, 1], F32, tag="w2c")
                nc.vector.tensor_mul(w2c, e2, inv)
                t1 = p_sm.tile([128, E], F32, tag="t1")
                nc.vector.tensor_scalar_mul(t1, eq1, inv)
                t2 = p_sm.tile([128, E], F32, tag="t2")
                nc.vector.tensor_scalar_mul(t2, eq2, w2c)
                nc.vector.tensor_add(W_s[:, mt_tok], t1, t2)
            # ---------- exchange: AllGather (u, W^T) ----------
            WT_sb = p_row.tile([8, T], F32, tag="wtsb")
            for mt_tok in range(2):
                pt = ppX.tile([128, TCH], F32, tag="x")
                nc.tensor.transpose(pt[:8, :128], W_s[:, mt_tok], ident[:])
                nc.scalar.copy(WT_sb[:, mt_tok * 128:(mt_tok + 1) * 128],
                               pt[:8, :128])
            nc.sync.dma_start(
                ag_in[:KT * 128, :].rearrange("(k p) t -> p k t", p=128), u)
            nc.sync.dma_start(ag_in[KT * 128:, :], WT_sb)
            nc.gpsimd.collective_compute(
                "AllGather", ALU.bypass, replica_groups=rg,
                ins=[ag_in.opt()], outs=[ag_out.opt()])
            WTe = p_row.tile([8, T], F32, tag="wte")
            for r in range(N_CORES):
                wrows = p_sm.tile([8, T], F32, tag="wrows")
                nc.sync.dma_start(wrows, ag_out[r, KT * 128:, :])
                ps_sel = ppS.tile([1, T], F32, tag="sm")
                nc.tensor.matmul(ps_sel, oh_s[:, :1], wrows, start=True, stop=True)
                selrow = p_sm.tile([1, T], F32, tag="selrow")
                nc.scalar.copy(selrow, ps_sel)
                nc.sync.dma_start(WTe[r:r + 1, :], selrow)
            # ---------- expert FFN (this core's expert) ----------
            w2_s = p_w2.tile([128, MTH, D], _dt("w2"), tag="w2")
            nc.sync.dma_start(
                w2_s, w2e_d.ap()[l].rearrange("(k p) m -> p k m", p=128))
            b1_s = p_row.tile([1, HFF], F32, tag="biasrow")
            nc.sync.dma_start(b1_s, b1e_d.ap()[l])
            b2_s = p_row.tile([1, D], F32, tag="biasrow2")
            nc.sync.dma_start(b2_s, b2e_d.ap()[l])
            for chk in range(NCH):
                ps_wb = ppX.tile([128, TCH], F32, tag="x")
                for rr2 in range(RPC):
                    r_g = chk * RPC + rr2
                    nc.tensor.matmul(ps_wb[:, rr2 * T:(rr2 + 1) * T],
                                     selm_s[:, r_g * 128:(r_g + 1) * 128],
                                     WTe, start=True, stop=True)
                wrow_s = p_row.tile([1, TCH], F32, tag="wrow")
                nc.scalar.copy(wrow_s, ps_wb[0:1, :])
                xw = p_xw.tile([128, KT, TCH], _dt("w1"), tag="vxw")
                for kt in range(KT):
                    xg = p_gth.tile([128, RPC, T], F32, tag="xg")
                    nc.sync.dma_start(
                        xg, ag_out[chk * RPC:(chk + 1) * RPC,
                                   kt * 128:(kt + 1) * 128, :]
                        .rearrange("r p t -> p r t"))
                    nc.vector.tensor_tensor(
                        xw[:, kt], xg.rearrange("p r t -> p (r t)"),
                        ps_wb, ALU.mult)
                h_s = p_big.tile([128, MTH, TCH], _dt("w2"), tag="big")
                for mt in range(MTH):
                    w1t = p_wst.tile([128, KT, 128], _dt("w1"), tag="wtile")
                    nc.sync.dma_start(
                        w1t, w1e_d.ap()[l].rearrange("(k p) m -> p k m", p=128)
                        [:, :, mt * 128:(mt + 1) * 128])
                    ps_h = ppMM.tile([128, TCH], F32, tag="mm")
                    for kt in range(KT):
                        nc.tensor.matmul(ps_h, w1t[:, kt],
                                         xw[:, kt],
                                         start=(kt == 0), stop=False)
                    nc.tensor.matmul(ps_h, b1_s[:, mt * 128:(mt + 1) * 128],
                                     wrow_s, start=False, stop=True)
                    nc.scalar.activation(out=h_s[:, mt], in_=ps_h, func=AF.Relu)
                for mt in range(KT):
                    ps_y = ppMM.tile([128, TCH], F32, tag="mm")
                    for kt in range(MTH):
                        nc.tensor.matmul(
                            ps_y, w2_s[:, kt, mt * 128:(mt + 1) * 128],
                            h_s[:, kt], start=(kt == 0), stop=False)
                    nc.tensor.matmul(ps_y, b2_s[:, mt * 128:(mt + 1) * 128],
                                     wrow_s, start=False, stop=True)
                    y_sb = p_attn.tile([128, TCH], F32, tag="ysb")
                    nc.vector.tensor_copy(y_sb, ps_y)
                    nc.sync.dma_start(
                        z_dram[chk * RPC:(chk + 1) * RPC, mt, :, :]
                        .rearrange("r p t -> p r t"), y_sb)
            nc.gpsimd.collective_compute(
                "ReduceScatter", ALU.add, replica_groups=rg,
                ins=[z_dram.opt()], outs=[rs_out.opt()])
            m_s = p_x.tile([128, KT, T], F32, tag="xbuf")
            nc.sync.dma_start(m_s, rs_out.rearrange("k p t -> p k t"))
            for kt in range(KT):
                nc.vector.tensor_add(u[:, kt], u[:, kt], m_s[:, kt])
            x_cur = layernorm(u, ln2g_d, ln2b_d, l)

        # ---------- pool + classifier ----------
        pool_c = p_sm.tile([128, KT], F32, tag="pool")
        for kt in range(KT):
            rsm = p_sm.tile([128, 1], F32, tag="psum1")
            nc.vector.reduce_sum(rsm, x_cur[:, kt], axis=mybir.AxisListType.X)
            nc.vector.tensor_scalar_mul(pool_c[:, kt:kt + 1], rsm, 1.0 / S)
        cls_s = p_sm.tile([128, KT, C], F32, tag="clsw")
        nc.sync.dma_start(cls_s, clsT_d.ap().rearrange("(k p) c -> p k c", p=128))
        ps_c = ppS.tile([C, 1], F32, tag="sm")
        for kt in range(KT):
            nc.tensor.matmul(ps_c, cls_s[:, kt], pool_c[:, kt:kt + 1],
                             start=(kt == 0), stop=False)
        nc.tensor.matmul(ps_c, clsb_s, one1, start=False, stop=True)
        cls_sb = p_sm.tile([C, 1], F32, tag="clso")
        nc.vector.tensor_copy(cls_sb, ps_c)
        nc.sync.dma_start(cls_out_d.ap(), cls_sb)

    nc.compile()
    return nc


_NC_CACHE = None


def kernel(**inputs):
    global _NC_CACHE
    inp = {k: np.asarray(v) for k, v in inputs.items()}
    pe = _pos_encoding_np()

    ipw = inp["in_proj_w"]
    wqkT = np.ascontiguousarray(ipw[:, :2 * D, :].transpose(0, 2, 1))
    wvT = np.ascontiguousarray(ipw[:, 2 * D:, :].transpose(0, 2, 1))
    ipb = inp["in_proj_b"]
    common = dict(
        emb=inp["emb"],
        wqkT=wqkT,
        bqk=ipb[:, :2 * D].reshape(L, 16, 128),
        wvT=wvT,
        bv=ipb[:, 2 * D:].reshape(L, 1, D),
        owT=inp["out_w"].transpose(0, 2, 1),
        ob=inp["out_b"].reshape(L, 1, D),
        ln1g=inp["ln1_g"].reshape(L, KT, 128),
        ln1b=inp["ln1_b"].reshape(L, KT, 128),
        ln2g=inp["ln2_g"].reshape(L, KT, 128),
        ln2b=inp["ln2_b"].reshape(L, KT, 128),
        gwT=inp["gate_w"].transpose(0, 2, 1),
        gb=inp["gate_b"].reshape(L, 1, E),
        clsT=inp["cls_w"].T,
        selm=np.kron(np.eye(8, dtype=np.float32), np.ones((1, 128), np.float32)),
        clsb=inp["cls_b"].reshape(1, C),
    )
    common = {k: np.ascontiguousarray(v, dtype=np.float32) for k, v in common.items()}

    in_maps = []
    for c in range(N_CORES):
        onehot = np.zeros((8, 1), np.float32)
        onehot[c, 0] = 1.0
        m = dict(common)
        m["srcl"] = np.ascontiguousarray(inp["src"][c].reshape(2, 128))
        m["pe"] = np.ascontiguousarray(pe[c].reshape(KT, 128))
        m["onehot"] = onehot
        m["w1e"] = np.ascontiguousarray(inp["w1"][:, c], dtype=np.float32)
        m["b1e"] = np.ascontiguousarray(inp["b1"][:, c].reshape(L, 1, HFF),
                                        dtype=np.float32)
        m["w2e"] = np.ascontiguousarray(inp["w2"][:, c], dtype=np.float32)
        m["b2e"] = np.ascontiguousarray(inp["b2"][:, c].reshape(L, 1, D),
                                        dtype=np.float32)
        in_maps.append(m)

    if _NC_CACHE is None:
        _NC_CACHE = build_kernel()
    nc = _NC_CACHE

    res = run_bass_kernel_spmd(nc, in_maps, core_ids=list(range(N_CORES)),
                               trace=bool(os.environ.get("KTRACE")))
    if os.environ.get("KTRACE"):
        kernel.last_exec_ns = res.exec_time_ns
        kernel.last_trace = res.instructions_and_trace

    out = np.stack([res.results[c]["cls_out"][:, 0] for c in range(N_CORES)])
    gates = np.stack([res.results[c]["gates_out"].reshape(L, S, E)
                      for c in range(N_CORES)], axis=1)
    return out, gates


# revision 26
# speedup vs baseline: 1.0707x; 1.0707x over previous
import contextlib
import math
import os

import numpy as np

import concourse.bass as bass
import concourse.mybir as mybir
import concourse.tile as tile
from concourse import bacc
from concourse.bass_utils import run_bass_kernel_spmd
from concourse.masks import make_identity

# Model dims (hardcoded for nn_MoETransformerClassifier_76433237999751)
B, S, D, NH, HFF, E, TOPK, C, L, V = 8, 256, 1024, 16, 2048, 8, 2, 10, 4, 32000
HD = D // NH          # 64
T = S                 # 256 local tokens per core
KT = D // 128         # 8 k-tiles over D
MTH = HFF // 128      # 16 m-tiles over HFF
N_CORES = 8
EPS = 1e-5
TCH = 512             # token chunk for expert FFN
NCH = (N_CORES * T) // TCH  # 4 chunks of global tokens
RPC = TCH // T        # ranks per chunk = 2

F32 = mybir.dt.float32
F32R = mybir.dt.float32r
AF = mybir.ActivationFunctionType
ALU = mybir.AluOpType

# Per-matmul-group precision: "f32r" (full PE rate, rel err ~1.5e-4) or "f32"
# (exact, 4 cyc/row). Gate/stat/bias/select matmuls are always fp32.
# qkv/v read the fp32 residual stream directly -> fp32 (mixed dtypes illegal).
# av uses tile_position col-tiling, which fp32r's weight path cannot encode.
PREC = dict(qkv="f32r", v="f32r", score="f32r", av="f32", oproj="f32r",
            w1="f32r", w2="f32r")
_env = os.environ.get("KPREC")
if _env:
    for kv in _env.split(","):
        k, vv = kv.split("=")
        if k == "all":
            for kk in PREC:
                PREC[kk] = vv
        else:
            PREC[k] = vv


def _dt(group):
    return F32R if PREC[group] == "f32r" else F32


def _pos_encoding_np():
    pos = np.arange(5000, dtype=np.float32)[:, None]
    div = np.exp(np.arange(0, D, 2, dtype=np.float32) * (-math.log(10000.0) / D))
    pe = np.zeros((5000, D), dtype=np.float32)
    pe[:, 0::2] = np.sin(pos * div)
    pe[:, 1::2] = np.cos(pos * div)
    return pe[:B]  # [B, D]


def build_kernel():
    nc = bacc.Bacc(None, target_bir_lowering=False, debug=False,
                   num_devices=N_CORES)

    def din(name, shape, dtype=F32):
        return nc.dram_tensor(name, shape, dtype, kind="ExternalInput")

    emb_d = din("emb", [V, D])
    srcl_d = din("srcl", [2, 128], mybir.dt.int32)
    pe_d = din("pe", [KT, 128])
    onehot_d = din("onehot", [8, 1])
    wqkT_d = din("wqkT", [L, D, 2 * D], _dt("qkv"))
    bqk_d = din("bqk", [L, 16, 128])
    wvT_d = din("wvT", [L, D, D], _dt("v"))
    bv_d = din("bv", [L, 1, D])
    owT_d = din("owT", [L, D, D], _dt("oproj"))
    ob_d = din("ob", [L, 1, D])
    ln1g_d = din("ln1g", [L, KT, 128])
    ln1b_d = din("ln1b", [L, KT, 128])
    ln2g_d = din("ln2g", [L, KT, 128])
    ln2b_d = din("ln2b", [L, KT, 128])
    gwT_d = din("gwT", [L, D, E])
    gb_d = din("gb", [L, 1, E])
    w1e_d = din("w1e", [L, D, HFF], _dt("w1"))
    b1e_d = din("b1e", [L, 1, HFF])
    w2e_d = din("w2e", [L, HFF, D], _dt("w2"))
    b2e_d = din("b2e", [L, 1, D])
    selm_d = din("selm", [8, 8 * 128])
    clsT_d = din("clsT", [D, C])
    clsb_d = din("clsb", [1, C])

    cls_out_d = nc.dram_tensor("cls_out", [C, 1], F32, kind="ExternalOutput")
    gates_out_d = nc.dram_tensor("gates_out", [L, 2, 128, E], F32,
                                 kind="ExternalOutput")

    rg = [list(range(N_CORES))]

    with tile.TileContext(nc) as tc, contextlib.ExitStack() as ctx:
        dram = ctx.enter_context(tc.tile_pool(name="dram", bufs=1, space="DRAM"))
        singles = ctx.enter_context(tc.tile_pool(name="singles", bufs=1))
        p_x = ctx.enter_context(tc.tile_pool(name="p_x", bufs=2))
        p_big = ctx.enter_context(tc.tile_pool(name="p_big", bufs=1))
        p_xw = ctx.enter_context(tc.tile_pool(name="p_xw", bufs=1))
        p_oT = ctx.enter_context(tc.tile_pool(name="p_oT", bufs=1))
        p_wst = ctx.enter_context(tc.tile_pool(name="p_wst", bufs=2))
        p_attn = ctx.enter_context(tc.tile_pool(name="p_attn", bufs=2))
        p_gth = ctx.enter_context(tc.tile_pool(name="p_gth", bufs=2))
        p_sm = ctx.enter_context(tc.tile_pool(name="p_sm", bufs=2))
        p_row = ctx.enter_context(tc.tile_pool(name="p_row", bufs=1))
        ppMM = ctx.enter_context(tc.tile_pool(name="ppMM", bufs=3, space="PSUM"))
        ppX = ctx.enter_context(tc.tile_pool(name="ppX", bufs=3, space="PSUM"))
        ppS = ctx.enter_context(tc.tile_pool(name="ppS", bufs=2, space="PSUM"))

        ident = singles.tile([128, 128], F32)
        make_identity(nc, ident[:])
        identr = singles.tile([128, 128], F32R)
        nc.vector.tensor_copy(identr, ident)
        ones_col = singles.tile([128, 1], F32)
        nc.vector.memset(ones_col, 1.0)
        ones_row = singles.tile([1, TCH], F32)
        nc.vector.memset(ones_row, 1.0)
        pe_s = singles.tile([128, KT], F32)
        nc.sync.dma_start(pe_s, pe_d.ap().rearrange("k p -> p k"))
        oh_s = singles.tile([8, 1], F32)
        nc.sync.dma_start(oh_s, onehot_d.ap())
        clsb_s = singles.tile([1, C], F32)
        nc.sync.dma_start(clsb_s, clsb_d.ap())
        one1 = singles.tile([1, 1], F32)
        nc.vector.memset(one1, 1.0)
        selm_s = singles.tile([8, 8 * 128], F32)
        nc.sync.dma_start(selm_s, selm_d.ap())
        eps_s = singles.tile([1, 1], F32)
        nc.vector.memset(eps_s, EPS)

        ag_in = dram.tile([KT * 128 + 8, T], F32)
        ag_outs = [dram.tile([N_CORES, KT * 128 + 8, T], F32,
                             addr_space="Shared", tag=f"ag{i}", name=f"ag_out{i}")
                   for i in range(L)]
        z_dram = dram.tile([N_CORES, KT, 128, T], F32)
        rs_out = dram.tile([KT, 128, T], F32)

        # ---------- embedding gather -> xT (feature-major) ----------
        xT = p_x.tile([128, KT, T], F32, tag="xbuf")
        for h in range(2):
            idx = p_sm.tile([128, 1], mybir.dt.int32, tag="idx")
            nc.sync.dma_start(idx, srcl_d.ap()[h].rearrange("(t o) -> t o", o=1))
            gt = p_xw.tile([128, D], F32, tag="vxw")
            nc.gpsimd.indirect_dma_start(
                out=gt[:], out_offset=None, in_=emb_d.ap(),
                in_offset=bass.IndirectOffsetOnAxis(ap=idx[:, :1], axis=0))
            for kt in range(KT):
                pt = ppX.tile([128, TCH], F32, tag="x")
                nc.tensor.transpose(pt[:, :128], gt[:, kt * 128:(kt + 1) * 128],
                                    ident[:])
                nc.scalar.activation(
                    out=xT[:, kt, h * 128:(h + 1) * 128], in_=pt[:, :128],
                    func=AF.Identity, scale=32.0, bias=pe_s[:, kt:kt + 1])

        def layernorm(xin, g_d, b_d, l):
            ps_sum = ppS.tile([1, T], F32, tag="sm")
            ps_sq = ppS.tile([1, T], F32, tag="sm")
            for kt in range(KT):
                sq_t = p_attn.tile([128, T], F32, tag="sqt")
                nc.scalar.activation(out=sq_t, in_=xin[:, kt], func=AF.Square)
                nc.tensor.matmul(ps_sum, ones_col[:], xin[:, kt],
                                 start=(kt == 0), stop=(kt == KT - 1))
                nc.tensor.matmul(ps_sq, ones_col[:], sq_t,
                                 start=(kt == 0), stop=(kt == KT - 1))
            mu = p_row.tile([1, T], F32, tag="rA")
            nc.scalar.activation(out=mu, in_=ps_sum, func=AF.Copy, scale=1.0 / D)
            ps_mu = ppX.tile([128, TCH], F32, tag="x")
            nc.tensor.matmul(ps_mu[:, :T], ones_row[:1, :128], mu,
                             start=True, stop=True)
            msq = p_row.tile([1, T], F32, tag="rB")
            nc.scalar.activation(out=msq, in_=ps_sq, func=AF.Copy, scale=1.0 / D)
            mu2 = p_row.tile([1, T], F32, tag="rC")
            nc.vector.tensor_mul(mu2, mu, mu)
            var = p_row.tile([1, T], F32, tag="rA")
            nc.vector.tensor_tensor(var, msq, mu2, ALU.subtract)
            vpe = p_row.tile([1, T], F32, tag="rB")
            nc.vector.tensor_scalar_add(vpe, var, EPS)
            lnv = p_row.tile([1, T], F32, tag="rC")
            nc.scalar.activation(out=lnv, in_=var, func=AF.Ln, bias=eps_s[:1])
            r0 = p_row.tile([1, T], F32, tag="rA")
            nc.scalar.activation(out=r0, in_=lnv, func=AF.Exp, scale=-0.5)
            # one Newton step: r' = r0*(1.5 - 0.5*(var+eps)*r0^2)
            rr = p_row.tile([1, T], F32, tag="rC")
            nc.vector.tensor_mul(rr, r0, r0)
            hv = p_row.tile([1, T], F32, tag="rD")
            nc.vector.tensor_scalar_mul(hv, vpe, -0.5)
            a1 = p_row.tile([1, T], F32, tag="rB")
            nc.vector.tensor_mul(a1, hv, rr)
            nc.vector.tensor_scalar_add(a1, a1, 1.5)
            rstd = p_row.tile([1, T], F32, tag="rC")
            nc.vector.tensor_mul(rstd, r0, a1)
            ps_rs = ppX.tile([128, TCH], F32, tag="x")
            nc.tensor.matmul(ps_rs[:, :T], ones_row[:1, :128], rstd,
                             start=True, stop=True)
            g_s = p_sm.tile([128, KT], F32, tag="lng")
            nc.sync.dma_start(g_s, g_d.ap()[l].rearrange("k p -> p k"))
            b_s = p_sm.tile([128, KT], F32, tag="lnb")
            nc.sync.dma_start(b_s, b_d.ap()[l].rearrange("k p -> p k"))
            xout = p_x.tile([128, KT, T], F32, tag="xbuf")
            for kt in range(KT):
                tt = p_attn.tile([128, T], F32, tag="lntmp")
                nc.vector.tensor_tensor(tt, xin[:, kt], ps_mu[:, :T], ALU.subtract)
                nc.vector.tensor_tensor(tt, tt, ps_rs[:, :T], ALU.mult)
                nc.scalar.activation(out=xout[:, kt], in_=tt, func=AF.Identity,
                                     scale=g_s[:, kt:kt + 1],
                                     bias=b_s[:, kt:kt + 1])
            return xout

        x_cur = xT
        for l in range(L):
            # fp32r view of the stream for attention input matmuls (the fp32
            # residual stream itself stays exact for routing)
            if PREC["qkv"] == "f32r":
                xr = p_x.tile([128, KT, T], F32R, tag="xr", bufs=1)
                for kt in range(KT):
                    nc.gpsimd.tensor_copy(out=xr[:, kt], in_=x_cur[:, kt])
            else:
                xr = x_cur
            # ---------- attention: QK (feature-major) ----------
            qk_s = p_big.tile([128, 16, T], _dt("score"), tag="big")
            bqk_s = p_sm.tile([128, 16], F32, tag="bqk")
            nc.sync.dma_start(bqk_s, bqk_d.ap()[l].rearrange("m p -> p m"))
            for mt in range(16):
                wt = p_wst.tile([128, KT, 128], _dt("qkv"), tag="wtile")
                nc.sync.dma_start(
                    wt, wqkT_d.ap()[l].rearrange("(k p) m -> p k m", p=128)
                    [:, :, mt * 128:(mt + 1) * 128])
                ps = ppMM.tile([128, TCH], F32, tag="mm")
                for kt in range(KT):
                    nc.tensor.matmul(ps[:, :T], wt[:, kt],
                                     xr[:, kt],
                                     start=(kt == 0), stop=(kt == KT - 1))
                nc.scalar.activation(out=qk_s[:, mt], in_=ps[:, :T],
                                     func=AF.Identity,
                                     bias=bqk_s[:, mt:mt + 1])
            # ---------- V (token-major) ----------
            v_s = p_xw.tile([128, KT, TCH], _dt("av"), tag="vxw")
            bv_s = p_row.tile([1, D], F32, tag="biasrow")
            nc.sync.dma_start(bv_s, bv_d.ap()[l])
            for mt_tok in range(2):
                ps0 = ppMM.tile([128, TCH], F32, tag="mm")
                ps1 = ppMM.tile([128, TCH], F32, tag="mm")
                pss = [ps0, ps1]
                for kt in range(KT):
                    vw = p_wst.tile([128, D], _dt("v"), tag="vw")
                    nc.sync.dma_start(vw, wvT_d.ap()[l][kt * 128:(kt + 1) * 128, :])
                    for nh2 in range(2):
                        nc.tensor.matmul(
                            pss[nh2],
                            xr[:, kt, mt_tok * 128:(mt_tok + 1) * 128],
                            vw[:, nh2 * 512:(nh2 + 1) * 512],
                            start=(kt == 0), stop=False)
                for nh2 in range(2):
                    nc.tensor.matmul(pss[nh2], ones_row[:1, :128],
                                     bv_s[:, nh2 * 512:(nh2 + 1) * 512],
                                     start=False, stop=True)
                    nc.scalar.copy(v_s[:, mt_tok * 2 + nh2, :], pss[nh2])
            # v layout: v_s[:, mt_tok*2 + nh2, c] = V[token tile mt_tok, feature nh2*512+c]
            # ---------- attention heads ----------
            oT_s = p_oT.tile([128, 8, T], _dt("oproj"), tag="oT")
            for h in range(NH):
                mt_q = h // 2
                r0 = (h % 2) * 64
                attnT = p_attn.tile([128, 2, T], _dt("av"), tag="attnT")
                for qh in range(2):
                    ps_sc = ppMM.tile([128, TCH], F32, tag="mm")
                    nc.tensor.matmul(
                        ps_sc[:, :T],
                        qk_s[r0:r0 + 64, mt_q, qh * 128:(qh + 1) * 128],
                        qk_s[r0:r0 + 64, 8 + mt_q, :],
                        start=True, stop=True)
                    mx = p_sm.tile([128, 1], F32, tag="mx")
                    nc.vector.reduce_max(mx, ps_sc[:, :T], axis=mybir.AxisListType.X)
                    nmx = p_sm.tile([128, 1], F32, tag="nmx")
                    nc.vector.tensor_scalar_mul(nmx, mx, -0.125)
                    esum = p_sm.tile([128, 1], F32, tag="esum")
                    ae = p_attn.tile([128, T], F32, tag="ae")
                    nc.scalar.activation(out=ae, in_=ps_sc[:, :T], func=AF.Exp,
                                         scale=0.125, bias=nmx, accum_out=esum)
                    rs = p_sm.tile([128, 1], F32, tag="rsum")
                    nc.vector.reciprocal(rs, esum)
                    an = p_attn.tile([128, T], _dt("av"), tag="an")
                    nc.vector.tensor_scalar_mul(an, ae, rs)
                    for kh in range(2):
                        pt = ppX.tile([128, TCH], _dt("av"), tag="x")
                        nc.tensor.transpose(
                            pt[:, :128], an[:, kh * 128:(kh + 1) * 128],
                            identr[:] if _dt("av") == F32R else ident[:])
                        nc.scalar.copy(attnT[:, kh, qh * 128:(qh + 1) * 128], pt[:, :128])
                if h % 2 == 0:
                    ps_o = ppMM.tile([128, TCH], F32, tag="mm")
                for kt2 in range(2):
                    # v slice for head h, token-tile kt2:
                    # d-range h*64:(h+1)*64 lives in nh2 = h//8, col h%8*64...
                    nh2 = (h * 64) // 512
                    c0 = (h * 64) % 512
                    vsl = v_s[:, kt2 * 2 + nh2, c0:c0 + 64]
                    nc.tensor.matmul(
                        ps_o[r0:r0 + 64, :T], vsl,
                        attnT[:, kt2, :],
                        start=(kt2 == 0), stop=(kt2 == 1),
                        tile_position=(0, r0))
                if h % 2 == 1:
                    nc.scalar.copy(oT_s[:, h // 2, :], ps_o[:, :T])
            # ---------- out-proj + residual ----------
            x2 = p_x.tile([128, KT, T], F32, tag="xbuf")
            ob_s = p_row.tile([1, D], F32, tag="biasrow")
            nc.sync.dma_start(ob_s, ob_d.ap()[l])
            for mt in range(KT):
                wt = p_wst.tile([128, KT, 128], _dt("oproj"), tag="wtile")
                nc.sync.dma_start(
                    wt, owT_d.ap()[l].rearrange("(k p) m -> p k m", p=128)
                    [:, :, mt * 128:(mt + 1) * 128])
                ps = ppMM.tile([128, TCH], F32, tag="mm")
                for kt in range(KT):
                    nc.tensor.matmul(ps[:, :T], wt[:, kt],
                                     oT_s[:, kt],
                                     start=(kt == 0), stop=False)
                nc.tensor.matmul(ps[:, :T],
                                 ob_s[:, mt * 128:(mt + 1) * 128],
                                 ones_row[:1, :T], start=False, stop=True)
                nc.vector.scalar_tensor_tensor(
                    out=x2[:, mt], in0=ps[:, :T], scalar=1.0, in1=x_cur[:, mt],
                    op0=ALU.mult, op1=ALU.add)
            u = layernorm(x2, ln1g_d, ln1b_d, l)
            # ---------- gate + top-2 ----------
            gw_s = p_sm.tile([128, KT, E], F32, tag="gw")
            nc.sync.dma_start(gw_s,
                              gwT_d.ap()[l].rearrange("(k p) e -> p k e", p=128))
            gb_s = p_row.tile([1, E], F32, tag="gb")
            nc.sync.dma_start(gb_s, gb_d.ap()[l])
            W_s = p_sm.tile([128, 2, E], F32, tag="Ws")
            for mt_tok in range(2):
                ps_g = ppS.tile([128, E], F32, tag="sm")
                for kt in range(KT):
                    nc.tensor.matmul(
                        ps_g, u[:, kt, mt_tok * 128:(mt_tok + 1) * 128],
                        gw_s[:, kt], start=(kt == 0), stop=False)
                nc.tensor.matmul(ps_g, ones_row[:1, :128], gb_s,
                                 start=False, stop=True)
                logit = p_sm.tile([128, E], F32, tag="logit")
                nc.vector.tensor_copy(logit, ps_g)
                nc.sync.dma_start(gates_out_d.ap()[l, mt_tok], logit)
                m1 = p_sm.tile([128, 1], F32, tag="m1")
                nc.vector.reduce_max(m1, logit, axis=mybir.AxisListType.X)
                eq1 = p_sm.tile([128, E], F32, tag="eq1")
                nc.vector.tensor_scalar(out=eq1, in0=logit, scalar1=m1,
                                        scalar2=None, op0=ALU.is_equal)
                msk = p_sm.tile([128, E], F32, tag="msk")
                nc.vector.scalar_tensor_tensor(
                    out=msk, in0=eq1, scalar=-1e30, in1=logit,
                    op0=ALU.mult, op1=ALU.add)
                m2 = p_sm.tile([128, 1], F32, tag="m2")
                nc.vector.reduce_max(m2, msk, axis=mybir.AxisListType.X)
                eq2 = p_sm.tile([128, E], F32, tag="eq2")
                nc.vector.tensor_scalar(out=eq2, in0=msk, scalar1=m2,
                                        scalar2=None, op0=ALU.is_equal)
                nm1 = p_sm.tile([128, 1], F32, tag="nm1")
                nc.vector.tensor_scalar_mul(nm1, m1, -1.0)
                e2 = p_sm.tile([128, 1], F32, tag="e2")
                nc.scalar.activation(out=e2, in_=m2, func=AF.Exp, bias=nm1)
                den = p_sm.tile([128, 1], F32, tag="den")
                nc.vector.tensor_scalar_add(den, e2, 1.0)
                inv = p_sm.tile([128, 1], F32, tag="inv")
                nc.vector.reciprocal(inv, den)
                w2c = p_sm.tile([128, 1], F32, tag="w2c")
                nc.vector.tensor_mul(w2c, e2, inv)
                t1 = p_sm.tile([128, E], F32, tag="t1")
                nc.vector.tensor_scalar_mul(t1, eq1, inv)
                t2 = p_sm.tile([128, E], F32, tag="t2")
                nc.vector.tensor_scalar_mul(t2, eq2, w2c)
                nc.vector.tensor_add(W_s[:, mt_tok], t1, t2)
            # ---------- exchange: AllGather (u, W^T) ----------
            WT_sb = p_row.tile([8, T], F32, tag="wtsb")
            for mt_tok in range(2):
                pt = ppX.tile([128, TCH], F32, tag="x")
                nc.tensor.transpose(pt[:8, :128], W_s[:, mt_tok], ident[:])
                nc.scalar.copy(WT_sb[:, mt_tok * 128:(mt_tok + 1) * 128],
                               pt[:8, :128])
            nc.sync.dma_start(
                ag_in[:KT * 128, :].rearrange("(k p) t -> p k t", p=128), u)
            nc.sync.dma_start(ag_in[KT * 128:, :], WT_sb)
            ag_out = ag_outs[l]
            nc.gpsimd.collective_compute(
                "AllGather", ALU.bypass, replica_groups=rg,
                ins=[ag_in.opt()], outs=[ag_out.opt()])
            WTe = p_row.tile([8, T], F32, tag="wte")
            for r in range(N_CORES):
                wrows = p_sm.tile([8, T], F32, tag="wrows")
                nc.sync.dma_start(wrows, ag_out[r, KT * 128:, :])
                ps_sel = ppS.tile([1, T], F32, tag="sm")
                nc.tensor.matmul(ps_sel, oh_s[:, :1], wrows, start=True, stop=True)
                selrow = p_sm.tile([1, T], F32, tag="selrow")
                nc.scalar.copy(selrow, ps_sel)
                nc.sync.dma_start(WTe[r:r + 1, :], selrow)
            # ---------- expert FFN (this core's expert) ----------
            b1_s = p_row.tile([1, HFF], F32, tag="biasrow")
            nc.sync.dma_start(b1_s, b1e_d.ap()[l])
            b2_s = p_row.tile([1, D], F32, tag="biasrow2")
            nc.sync.dma_start(b2_s, b2e_d.ap()[l])
            for chk in range(NCH):
                ps_wb = ppX.tile([128, TCH], F32, tag="x")
                for rr2 in range(RPC):
                    r_g = chk * RPC + rr2
                    nc.tensor.matmul(ps_wb[:, rr2 * T:(rr2 + 1) * T],
                                     selm_s[:, r_g * 128:(r_g + 1) * 128],
                                     WTe, start=True, stop=True)
                wrow_s = p_row.tile([1, TCH], F32, tag="wrow")
                nc.scalar.copy(wrow_s, ps_wb[0:1, :])
                xw = p_xw.tile([128, KT, TCH], _dt("w1"), tag="vxw")
                for kt in range(KT):
                    xg = p_gth.tile([128, RPC, T], F32, tag="xg")
                    nc.sync.dma_start(
                        xg, ag_out[chk * RPC:(chk + 1) * RPC,
                                   kt * 128:(kt + 1) * 128, :]
                        .rearrange("r p t -> p r t"))
                    nc.vector.tensor_tensor(
                        xw[:, kt], xg.rearrange("p r t -> p (r t)"),
                        ps_wb, ALU.mult)
                h_s = p_big.tile([128, MTH, TCH], _dt("w2"), tag="big")
                for mt in range(MTH):
                    w1t = p_wst.tile([128, KT, 128], _dt("w1"), tag="wtile")
                    nc.sync.dma_start(
                        w1t, w1e_d.ap()[l].rearrange("(k p) m -> p k m", p=128)
                        [:, :, mt * 128:(mt + 1) * 128])
                    ps_h = ppMM.tile([128, TCH], F32, tag="mm")
                    for kt in range(KT):
                        nc.tensor.matmul(ps_h, w1t[:, kt],
                                         xw[:, kt],
                                         start=(kt == 0), stop=False)
                    nc.tensor.matmul(ps_h, b1_s[:, mt * 128:(mt + 1) * 128],
                                     wrow_s, start=False, stop=True)
                    nc.scalar.activation(out=h_s[:, mt], in_=ps_h, func=AF.Relu)
                for mt in range(KT):
                    w2t = p_wst.tile([128, MTH, 128], _dt("w2"), tag="w2t")
                    nc.sync.dma_start(
                        w2t, w2e_d.ap()[l].rearrange("(k p) m -> p k m", p=128)
                        [:, :, mt * 128:(mt + 1) * 128])
                    ps_y = ppMM.tile([128, TCH], F32, tag="mm")
                    for kt in range(MTH):
                        nc.tensor.matmul(
                            ps_y, w2t[:, kt],
                            h_s[:, kt], start=(kt == 0), stop=False)
                    nc.tensor.matmul(ps_y, b2_s[:, mt * 128:(mt + 1) * 128],
                                     wrow_s, start=False, stop=True)
                    y_sb = p_attn.tile([128, TCH], F32, tag="ysb")
                    nc.vector.tensor_copy(y_sb, ps_y)
                    nc.sync.dma_start(
                        z_dram[chk * RPC:(chk + 1) * RPC, mt, :, :]
                        .rearrange("r p t -> p r t"), y_sb)
            nc.gpsimd.collective_compute(
                "ReduceScatter", ALU.add, replica_groups=rg,
                ins=[z_dram.opt()], outs=[rs_out.opt()])
            m_s = p_x.tile([128, KT, T], F32, tag="xbuf")
            nc.sync.dma_start(m_s, rs_out.rearrange("k p t -> p k t"))
            for kt in range(KT):
                nc.vector.tensor_add(u[:, kt], u[:, kt], m_s[:, kt])
            x_cur = layernorm(u, ln2g_d, ln2b_d, l)

        # ---------- pool + classifier ----------
        pool_c = p_sm.tile([128, KT], F32, tag="pool")
        for kt in range(KT):
            rsm = p_sm.tile([128, 1], F32, tag="psum1")
            nc.vector.reduce_sum(rsm, x_cur[:, kt], axis=mybir.AxisListType.X)
            nc.vector.tensor_scalar_mul(pool_c[:, kt:kt + 1], rsm, 1.0 / S)
        cls_s = p_sm.tile([128, KT, C], F32, tag="clsw")
        nc.sync.dma_start(cls_s, clsT_d.ap().rearrange("(k p) c -> p k c", p=128))
        ps_c = ppS.tile([C, 1], F32, tag="sm")
        for kt in range(KT):
            nc.tensor.matmul(ps_c, cls_s[:, kt], pool_c[:, kt:kt + 1],
                             start=(kt == 0), stop=False)
        nc.tensor.matmul(ps_c, clsb_s, one1, start=False, stop=True)
        cls_sb = p_sm.tile([C, 1], F32, tag="clso")
        nc.vector.tensor_copy(cls_sb, ps_c)
        nc.sync.dma_start(cls_out_d.ap(), cls_sb)

    nc.compile()
    return nc


_NC_CACHE = None


def kernel(**inputs):
    global _NC_CACHE
    inp = {k: np.asarray(v) for k, v in inputs.items()}
    pe = _pos_encoding_np()

    ipw = inp["in_proj_w"]
    wqkT = np.ascontiguousarray(ipw[:, :2 * D, :].transpose(0, 2, 1))
    wvT = np.ascontiguousarray(ipw[:, 2 * D:, :].transpose(0, 2, 1))
    ipb = inp["in_proj_b"]
    common = dict(
        emb=inp["emb"],
        wqkT=wqkT,
        bqk=ipb[:, :2 * D].reshape(L, 16, 128),
        wvT=wvT,
        bv=ipb[:, 2 * D:].reshape(L, 1, D),
        owT=inp["out_w"].transpose(0, 2, 1),
        ob=inp["out_b"].reshape(L, 1, D),
        ln1g=inp["ln1_g"].reshape(L, KT, 128),
        ln1b=inp["ln1_b"].reshape(L, KT, 128),
        ln2g=inp["ln2_g"].reshape(L, KT, 128),
        ln2b=inp["ln2_b"].reshape(L, KT, 128),
        gwT=inp["gate_w"].transpose(0, 2, 1),
        gb=inp["gate_b"].reshape(L, 1, E),
        clsT=inp["cls_w"].T,
        selm=np.kron(np.eye(8, dtype=np.float32), np.ones((1, 128), np.float32)),
        clsb=inp["cls_b"].reshape(1, C),
    )
    common = {k: np.ascontiguousarray(v, dtype=np.float32) for k, v in common.items()}

    in_maps = []
    for c in range(N_CORES):
        onehot = np.zeros((8, 1), np.float32)
        onehot[c, 0] = 1.0
        m = dict(common)
        m["srcl"] = np.ascontiguousarray(inp["src"][c].reshape(2, 128))
        m["pe"] = np.ascontiguousarray(pe[c].reshape(KT, 128))
        m["onehot"] = onehot
        m["w1e"] = np.ascontiguousarray(inp["w1"][:, c], dtype=np.float32)
        m["b1e"] = np.ascontiguousarray(inp["b1"][:, c].reshape(L, 1, HFF),
                                        dtype=np.float32)
        m["w2e"] = np.ascontiguousarray(inp["w2"][:, c], dtype=np.float32)
        m["b2e"] = np.ascontiguousarray(inp["b2"][:, c].reshape(L, 1, D),
                                        dtype=np.float32)
        in_maps.append(m)

    if _NC_CACHE is None:
        _NC_CACHE = build_kernel()
    nc = _NC_CACHE

    res = run_bass_kernel_spmd(nc, in_maps, core_ids=list(range(N_CORES)),
                               trace=bool(os.environ.get("KTRACE")))
    if os.environ.get("KTRACE"):
        kernel.last_exec_ns = res.exec_time_ns
        kernel.last_trace = res.instructions_and_trace

    out = np.stack([res.results[c]["cls_out"][:, 0] for c in range(N_CORES)])
    gates = np.stack([res.results[c]["gates_out"].reshape(L, S, E)
                      for c in range(N_CORES)], axis=1)
    return out, gates


# revision 27
# speedup vs baseline: 1.1031x; 1.0303x over previous
import contextlib
import math
import os

import numpy as np

import concourse.bass as bass
import concourse.mybir as mybir
import concourse.tile as tile
from concourse import bacc
from concourse.bass_utils import run_bass_kernel_spmd
from concourse.masks import make_identity

# Model dims (hardcoded for nn_MoETransformerClassifier_76433237999751)
B, S, D, NH, HFF, E, TOPK, C, L, V = 8, 256, 1024, 16, 2048, 8, 2, 10, 4, 32000
HD = D // NH          # 64
T = S                 # 256 local tokens per core
KT = D // 128         # 8 k-tiles over D
MTH = HFF // 128      # 16 m-tiles over HFF
N_CORES = 8
EPS = 1e-5
TCH = 512             # token chunk for expert FFN
NCH = (N_CORES * T) // TCH  # 4 chunks of global tokens
RPC = TCH // T        # ranks per chunk = 2

F32 = mybir.dt.float32
F32R = mybir.dt.float32r
AF = mybir.ActivationFunctionType
ALU = mybir.AluOpType

# Per-matmul-group precision: "f32r" (full PE rate, rel err ~1.5e-4) or "f32"
# (exact, 4 cyc/row). Gate/stat/bias/select matmuls are always fp32.
# qkv/v read the fp32 residual stream directly -> fp32 (mixed dtypes illegal).
# av uses tile_position col-tiling, which fp32r's weight path cannot encode.
PREC = dict(qkv="f32r", v="f32r", score="f32r", av="f32", oproj="f32r",
            w1="f32r", w2="f32r")
_env = os.environ.get("KPREC")
if _env:
    for kv in _env.split(","):
        k, vv = kv.split("=")
        if k == "all":
            for kk in PREC:
                PREC[kk] = vv
        else:
            PREC[k] = vv


def _dt(group):
    return F32R if PREC[group] == "f32r" else F32


def _pos_encoding_np():
    pos = np.arange(5000, dtype=np.float32)[:, None]
    div = np.exp(np.arange(0, D, 2, dtype=np.float32) * (-math.log(10000.0) / D))
    pe = np.zeros((5000, D), dtype=np.float32)
    pe[:, 0::2] = np.sin(pos * div)
    pe[:, 1::2] = np.cos(pos * div)
    return pe[:B]  # [B, D]


def build_kernel():
    nc = bacc.Bacc(None, target_bir_lowering=False, debug=False,
                   num_devices=N_CORES)

    def din(name, shape, dtype=F32):
        return nc.dram_tensor(name, shape, dtype, kind="ExternalInput")

    emb_d = din("emb", [V, D])
    srcl_d = din("srcl", [2, 128], mybir.dt.int32)
    pe_d = din("pe", [KT, 128])
    onehot_d = din("onehot", [8, 1])
    wqkT_d = din("wqkT", [L, 16, 128, KT, 128], _dt("qkv"))
    bqk_d = din("bqk", [L, 16, 128])
    wvT_d = din("wvT", [L, D, D], _dt("v"))
    bv_d = din("bv", [L, 1, D])
    owT_d = din("owT", [L, KT, 128, KT, 128], _dt("oproj"))
    ob_d = din("ob", [L, 1, D])
    ln1g_d = din("ln1g", [L, KT, 128])
    ln1b_d = din("ln1b", [L, KT, 128])
    ln2g_d = din("ln2g", [L, KT, 128])
    ln2b_d = din("ln2b", [L, KT, 128])
    gwT_d = din("gwT", [L, D, E])
    gb_d = din("gb", [L, 1, E])
    w1e_d = din("w1e", [L, MTH, 128, KT, 128], _dt("w1"))
    b1e_d = din("b1e", [L, 1, HFF])
    w2e_d = din("w2e", [L, KT, 128, MTH, 128], _dt("w2"))
    b2e_d = din("b2e", [L, 1, D])
    selm_d = din("selm", [8, 8 * 128])
    clsT_d = din("clsT", [D, C])
    clsb_d = din("clsb", [1, C])

    cls_out_d = nc.dram_tensor("cls_out", [C, 1], F32, kind="ExternalOutput")
    gates_out_d = nc.dram_tensor("gates_out", [L, 2, 128, E], F32,
                                 kind="ExternalOutput")

    rg = [list(range(N_CORES))]

    with tile.TileContext(nc) as tc, contextlib.ExitStack() as ctx:
        dram = ctx.enter_context(tc.tile_pool(name="dram", bufs=1, space="DRAM"))
        singles = ctx.enter_context(tc.tile_pool(name="singles", bufs=1))
        p_x = ctx.enter_context(tc.tile_pool(name="p_x", bufs=2))
        p_big = ctx.enter_context(tc.tile_pool(name="p_big", bufs=1))
        p_xw = ctx.enter_context(tc.tile_pool(name="p_xw", bufs=1))
        p_oT = ctx.enter_context(tc.tile_pool(name="p_oT", bufs=1))
        p_wst = ctx.enter_context(tc.tile_pool(name="p_wst", bufs=2))
        p_attn = ctx.enter_context(tc.tile_pool(name="p_attn", bufs=2))
        p_gth = ctx.enter_context(tc.tile_pool(name="p_gth", bufs=2))
        p_sm = ctx.enter_context(tc.tile_pool(name="p_sm", bufs=2))
        p_row = ctx.enter_context(tc.tile_pool(name="p_row", bufs=1))
        ppMM = ctx.enter_context(tc.tile_pool(name="ppMM", bufs=3, space="PSUM"))
        ppX = ctx.enter_context(tc.tile_pool(name="ppX", bufs=3, space="PSUM"))
        ppS = ctx.enter_context(tc.tile_pool(name="ppS", bufs=2, space="PSUM"))

        ident = singles.tile([128, 128], F32)
        make_identity(nc, ident[:])
        identr = singles.tile([128, 128], F32R)
        nc.vector.tensor_copy(identr, ident)
        ones_col = singles.tile([128, 1], F32)
        nc.vector.memset(ones_col, 1.0)
        ones_row = singles.tile([1, TCH], F32)
        nc.vector.memset(ones_row, 1.0)
        pe_s = singles.tile([128, KT], F32)
        nc.sync.dma_start(pe_s, pe_d.ap().rearrange("k p -> p k"))
        oh_s = singles.tile([8, 1], F32)
        nc.sync.dma_start(oh_s, onehot_d.ap())
        clsb_s = singles.tile([1, C], F32)
        nc.sync.dma_start(clsb_s, clsb_d.ap())
        one1 = singles.tile([1, 1], F32)
        nc.vector.memset(one1, 1.0)
        selm_s = singles.tile([8, 8 * 128], F32)
        nc.sync.dma_start(selm_s, selm_d.ap())
        eps_s = singles.tile([1, 1], F32)
        nc.vector.memset(eps_s, EPS)

        ag_in = dram.tile([KT * 128 + 8, T], F32)
        ag_outs = [dram.tile([N_CORES, KT * 128 + 8, T], F32,
                             addr_space="Shared", tag=f"ag{i}", name=f"ag_out{i}")
                   for i in range(L)]
        z_dram = dram.tile([N_CORES, KT, 128, T], F32)
        rs_out = dram.tile([KT, 128, T], F32)

        # ---------- embedding gather -> xT (feature-major) ----------
        xT = p_x.tile([128, KT, T], F32, tag="xbuf")
        for h in range(2):
            idx = p_sm.tile([128, 1], mybir.dt.int32, tag="idx")
            nc.sync.dma_start(idx, srcl_d.ap()[h].rearrange("(t o) -> t o", o=1))
            gt = p_xw.tile([128, D], F32, tag="vxw")
            nc.gpsimd.indirect_dma_start(
                out=gt[:], out_offset=None, in_=emb_d.ap(),
                in_offset=bass.IndirectOffsetOnAxis(ap=idx[:, :1], axis=0))
            for kt in range(KT):
                pt = ppX.tile([128, TCH], F32, tag="x")
                nc.tensor.transpose(pt[:, :128], gt[:, kt * 128:(kt + 1) * 128],
                                    ident[:])
                nc.scalar.activation(
                    out=xT[:, kt, h * 128:(h + 1) * 128], in_=pt[:, :128],
                    func=AF.Identity, scale=32.0, bias=pe_s[:, kt:kt + 1])

        def layernorm(xin, g_d, b_d, l):
            ps_sum = ppS.tile([1, T], F32, tag="sm")
            ps_sq = ppS.tile([1, T], F32, tag="sm")
            for kt in range(KT):
                sq_t = p_attn.tile([128, T], F32, tag="sqt")
                nc.scalar.activation(out=sq_t, in_=xin[:, kt], func=AF.Square)
                nc.tensor.matmul(ps_sum, ones_col[:], xin[:, kt],
                                 start=(kt == 0), stop=(kt == KT - 1))
                nc.tensor.matmul(ps_sq, ones_col[:], sq_t,
                                 start=(kt == 0), stop=(kt == KT - 1))
            mu = p_row.tile([1, T], F32, tag="rA")
            nc.scalar.activation(out=mu, in_=ps_sum, func=AF.Copy, scale=1.0 / D)
            ps_mu = ppX.tile([128, TCH], F32, tag="x")
            nc.tensor.matmul(ps_mu[:, :T], ones_row[:1, :128], mu,
                             start=True, stop=True)
            msq = p_row.tile([1, T], F32, tag="rB")
            nc.scalar.activation(out=msq, in_=ps_sq, func=AF.Copy, scale=1.0 / D)
            mu2 = p_row.tile([1, T], F32, tag="rC")
            nc.vector.tensor_mul(mu2, mu, mu)
            var = p_row.tile([1, T], F32, tag="rA")
            nc.vector.tensor_tensor(var, msq, mu2, ALU.subtract)
            vpe = p_row.tile([1, T], F32, tag="rB")
            nc.vector.tensor_scalar_add(vpe, var, EPS)
            lnv = p_row.tile([1, T], F32, tag="rC")
            nc.scalar.activation(out=lnv, in_=var, func=AF.Ln, bias=eps_s[:1])
            r0 = p_row.tile([1, T], F32, tag="rA")
            nc.scalar.activation(out=r0, in_=lnv, func=AF.Exp, scale=-0.5)
            # one Newton step: r' = r0*(1.5 - 0.5*(var+eps)*r0^2)
            rr = p_row.tile([1, T], F32, tag="rC")
            nc.vector.tensor_mul(rr, r0, r0)
            hv = p_row.tile([1, T], F32, tag="rD")
            nc.vector.tensor_scalar_mul(hv, vpe, -0.5)
            a1 = p_row.tile([1, T], F32, tag="rB")
            nc.vector.tensor_mul(a1, hv, rr)
            nc.vector.tensor_scalar_add(a1, a1, 1.5)
            rstd = p_row.tile([1, T], F32, tag="rC")
            nc.vector.tensor_mul(rstd, r0, a1)
            ps_rs = ppX.tile([128, TCH], F32, tag="x")
            nc.tensor.matmul(ps_rs[:, :T], ones_row[:1, :128], rstd,
                             start=True, stop=True)
            g_s = p_sm.tile([128, KT], F32, tag="lng")
            nc.sync.dma_start(g_s, g_d.ap()[l].rearrange("k p -> p k"))
            b_s = p_sm.tile([128, KT], F32, tag="lnb")
            nc.sync.dma_start(b_s, b_d.ap()[l].rearrange("k p -> p k"))
            xout = p_x.tile([128, KT, T], F32, tag="xbuf")
            for kt in range(KT):
                tt = p_attn.tile([128, T], F32, tag="lntmp")
                nc.vector.tensor_tensor(tt, xin[:, kt], ps_mu[:, :T], ALU.subtract)
                nc.vector.tensor_tensor(tt, tt, ps_rs[:, :T], ALU.mult)
                nc.scalar.activation(out=xout[:, kt], in_=tt, func=AF.Identity,
                                     scale=g_s[:, kt:kt + 1],
                                     bias=b_s[:, kt:kt + 1])
            return xout

        x_cur = xT
        for l in range(L):
            # fp32r view of the stream for attention input matmuls (the fp32
            # residual stream itself stays exact for routing)
            if PREC["qkv"] == "f32r":
                xr = p_x.tile([128, KT, T], F32R, tag="xr", bufs=1)
                for kt in range(KT):
                    nc.gpsimd.tensor_copy(out=xr[:, kt], in_=x_cur[:, kt])
            else:
                xr = x_cur
            # ---------- attention: QK (feature-major) ----------
            qk_s = p_big.tile([128, 16, T], _dt("score"), tag="big")
            bqk_s = p_sm.tile([128, 16], F32, tag="bqk")
            nc.sync.dma_start(bqk_s, bqk_d.ap()[l].rearrange("m p -> p m"))
            for mt in range(16):
                wt = p_wst.tile([128, KT, 128], _dt("qkv"), tag="wtile")
                nc.sync.dma_start(wt, wqkT_d.ap()[l, mt])
                ps = ppMM.tile([128, TCH], F32, tag="mm")
                for kt in range(KT):
                    nc.tensor.matmul(ps[:, :T], wt[:, kt],
                                     xr[:, kt],
                                     start=(kt == 0), stop=(kt == KT - 1))
                nc.scalar.activation(out=qk_s[:, mt], in_=ps[:, :T],
                                     func=AF.Identity,
                                     bias=bqk_s[:, mt:mt + 1])
            # ---------- V (token-major) ----------
            v_s = p_xw.tile([128, KT, TCH], _dt("av"), tag="vxw")
            bv_s = p_row.tile([1, D], F32, tag="biasrow")
            nc.sync.dma_start(bv_s, bv_d.ap()[l])
            for mt_tok in range(2):
                ps0 = ppMM.tile([128, TCH], F32, tag="mm")
                ps1 = ppMM.tile([128, TCH], F32, tag="mm")
                pss = [ps0, ps1]
                for kt in range(KT):
                    vw = p_wst.tile([128, D], _dt("v"), tag="vw")
                    nc.sync.dma_start(vw, wvT_d.ap()[l][kt * 128:(kt + 1) * 128, :])
                    for nh2 in range(2):
                        nc.tensor.matmul(
                            pss[nh2],
                            xr[:, kt, mt_tok * 128:(mt_tok + 1) * 128],
                            vw[:, nh2 * 512:(nh2 + 1) * 512],
                            start=(kt == 0), stop=False)
                for nh2 in range(2):
                    nc.tensor.matmul(pss[nh2], ones_row[:1, :128],
                                     bv_s[:, nh2 * 512:(nh2 + 1) * 512],
                                     start=False, stop=True)
                    nc.scalar.copy(v_s[:, mt_tok * 2 + nh2, :], pss[nh2])
            # v layout: v_s[:, mt_tok*2 + nh2, c] = V[token tile mt_tok, feature nh2*512+c]
            # ---------- attention heads ----------
            oT_s = p_oT.tile([128, 8, T], _dt("oproj"), tag="oT")
            for h in range(NH):
                mt_q = h // 2
                r0 = (h % 2) * 64
                attnT = p_attn.tile([128, 2, T], _dt("av"), tag="attnT")
                for qh in range(2):
                    ps_sc = ppMM.tile([128, TCH], F32, tag="mm")
                    nc.tensor.matmul(
                        ps_sc[:, :T],
                        qk_s[r0:r0 + 64, mt_q, qh * 128:(qh + 1) * 128],
                        qk_s[r0:r0 + 64, 8 + mt_q, :],
                        start=True, stop=True)
                    mx = p_sm.tile([128, 1], F32, tag="mx")
                    nc.vector.reduce_max(mx, ps_sc[:, :T], axis=mybir.AxisListType.X)
                    nmx = p_sm.tile([128, 1], F32, tag="nmx")
                    nc.vector.tensor_scalar_mul(nmx, mx, -0.125)
                    esum = p_sm.tile([128, 1], F32, tag="esum")
                    ae = p_attn.tile([128, T], F32, tag="ae")
                    nc.scalar.activation(out=ae, in_=ps_sc[:, :T], func=AF.Exp,
                                         scale=0.125, bias=nmx, accum_out=esum)
                    rs = p_sm.tile([128, 1], F32, tag="rsum")
                    nc.vector.reciprocal(rs, esum)
                    an = p_attn.tile([128, T], _dt("av"), tag="an")
                    nc.vector.tensor_scalar_mul(an, ae, rs)
                    for kh in range(2):
                        pt = ppX.tile([128, TCH], _dt("av"), tag="x")
                        nc.tensor.transpose(
                            pt[:, :128], an[:, kh * 128:(kh + 1) * 128],
                            identr[:] if _dt("av") == F32R else ident[:])
                        nc.scalar.copy(attnT[:, kh, qh * 128:(qh + 1) * 128], pt[:, :128])
                if h % 2 == 0:
                    ps_o = ppMM.tile([128, TCH], F32, tag="mm")
                for kt2 in range(2):
                    # v slice for head h, token-tile kt2:
                    # d-range h*64:(h+1)*64 lives in nh2 = h//8, col h%8*64...
                    nh2 = (h * 64) // 512
                    c0 = (h * 64) % 512
                    vsl = v_s[:, kt2 * 2 + nh2, c0:c0 + 64]
                    nc.tensor.matmul(
                        ps_o[r0:r0 + 64, :T], vsl,
                        attnT[:, kt2, :],
                        start=(kt2 == 0), stop=(kt2 == 1),
                        tile_position=(0, r0))
                if h % 2 == 1:
                    nc.scalar.copy(oT_s[:, h // 2, :], ps_o[:, :T])
            # ---------- out-proj + residual ----------
            x2 = p_x.tile([128, KT, T], F32, tag="xbuf")
            ob_s = p_row.tile([1, D], F32, tag="biasrow")
            nc.sync.dma_start(ob_s, ob_d.ap()[l])
            for mt in range(KT):
                wt = p_wst.tile([128, KT, 128], _dt("oproj"), tag="wtile")
                nc.sync.dma_start(wt, owT_d.ap()[l, mt])
                ps = ppMM.tile([128, TCH], F32, tag="mm")
                for kt in range(KT):
                    nc.tensor.matmul(ps[:, :T], wt[:, kt],
                                     oT_s[:, kt],
                                     start=(kt == 0), stop=False)
                nc.tensor.matmul(ps[:, :T],
                                 ob_s[:, mt * 128:(mt + 1) * 128],
                                 ones_row[:1, :T], start=False, stop=True)
                nc.vector.scalar_tensor_tensor(
                    out=x2[:, mt], in0=ps[:, :T], scalar=1.0, in1=x_cur[:, mt],
                    op0=ALU.mult, op1=ALU.add)
            u = layernorm(x2, ln1g_d, ln1b_d, l)
            # ---------- gate + top-2 ----------
            gw_s = p_sm.tile([128, KT, E], F32, tag="gw")
            nc.sync.dma_start(gw_s,
                              gwT_d.ap()[l].rearrange("(k p) e -> p k e", p=128))
            gb_s = p_row.tile([1, E], F32, tag="gb")
            nc.sync.dma_start(gb_s, gb_d.ap()[l])
            W_s = p_sm.tile([128, 2, E], F32, tag="Ws")
            for mt_tok in range(2):
                ps_g = ppS.tile([128, E], F32, tag="sm")
                for kt in range(KT):
                    nc.tensor.matmul(
                        ps_g, u[:, kt, mt_tok * 128:(mt_tok + 1) * 128],
                        gw_s[:, kt], start=(kt == 0), stop=False)
                nc.tensor.matmul(ps_g, ones_row[:1, :128], gb_s,
                                 start=False, stop=True)
                logit = p_sm.tile([128, E], F32, tag="logit")
                nc.vector.tensor_copy(logit, ps_g)
                nc.sync.dma_start(gates_out_d.ap()[l, mt_tok], logit)
                m1 = p_sm.tile([128, 1], F32, tag="m1")
                nc.vector.reduce_max(m1, logit, axis=mybir.AxisListType.X)
                eq1 = p_sm.tile([128, E], F32, tag="eq1")
                nc.vector.tensor_scalar(out=eq1, in0=logit, scalar1=m1,
                                        scalar2=None, op0=ALU.is_equal)
                msk = p_sm.tile([128, E], F32, tag="msk")
                nc.vector.scalar_tensor_tensor(
                    out=msk, in0=eq1, scalar=-1e30, in1=logit,
                    op0=ALU.mult, op1=ALU.add)
                m2 = p_sm.tile([128, 1], F32, tag="m2")
                nc.vector.reduce_max(m2, msk, axis=mybir.AxisListType.X)
                eq2 = p_sm.tile([128, E], F32, tag="eq2")
                nc.vector.tensor_scalar(out=eq2, in0=msk, scalar1=m2,
                                        scalar2=None, op0=ALU.is_equal)
                nm1 = p_sm.tile([128, 1], F32, tag="nm1")
                nc.vector.tensor_scalar_mul(nm1, m1, -1.0)
                e2 = p_sm.tile([128, 1], F32, tag="e2")
                nc.scalar.activation(out=e2, in_=m2, func=AF.Exp, bias=nm1)
                den = p_sm.tile([128, 1], F32, tag="den")
                nc.vector.tensor_scalar_add(den, e2, 1.0)
                inv = p_sm.tile([128, 1], F32, tag="inv")
                nc.vector.reciprocal(inv, den)
                w2c = p_sm.tile([128, 1], F32, tag="w2c")
                nc.vector.tensor_mul(w2c, e2, inv)
                t1 = p_sm.tile([128, E], F32, tag="t1")
                nc.vector.tensor_scalar_mul(t1, eq1, inv)
                t2 = p_sm.tile([128, E], F32, tag="t2")
                nc.vector.tensor_scalar_mul(t2, eq2, w2c)
                nc.vector.tensor_add(W_s[:, mt_tok], t1, t2)
            # ---------- exchange: AllGather (u, W^T) ----------
            WT_sb = p_row.tile([8, T], F32, tag="wtsb")
            for mt_tok in range(2):
                pt = ppX.tile([128, TCH], F32, tag="x")
                nc.tensor.transpose(pt[:8, :128], W_s[:, mt_tok], ident[:])
                nc.scalar.copy(WT_sb[:, mt_tok * 128:(mt_tok + 1) * 128],
                               pt[:8, :128])
            nc.sync.dma_start(
                ag_in[:KT * 128, :].rearrange("(k p) t -> p k t", p=128), u)
            nc.sync.dma_start(ag_in[KT * 128:, :], WT_sb)
            ag_out = ag_outs[l]
            nc.gpsimd.collective_compute(
                "AllGather", ALU.bypass, replica_groups=rg,
                ins=[ag_in.opt()], outs=[ag_out.opt()])
            WTe = p_row.tile([8, T], F32, tag="wte")
            for r in range(N_CORES):
                wrows = p_sm.tile([8, T], F32, tag="wrows")
                nc.sync.dma_start(wrows, ag_out[r, KT * 128:, :])
                ps_sel = ppS.tile([1, T], F32, tag="sm")
                nc.tensor.matmul(ps_sel, oh_s[:, :1], wrows, start=True, stop=True)
                selrow = p_sm.tile([1, T], F32, tag="selrow")
                nc.scalar.copy(selrow, ps_sel)
                nc.sync.dma_start(WTe[r:r + 1, :], selrow)
            # ---------- expert FFN (this core's expert) ----------
            b1_s = p_row.tile([1, HFF], F32, tag="biasrow")
            nc.sync.dma_start(b1_s, b1e_d.ap()[l])
            b2_s = p_row.tile([1, D], F32, tag="biasrow2")
            nc.sync.dma_start(b2_s, b2e_d.ap()[l])
            for chk in range(NCH):
                ps_wb = ppX.tile([128, TCH], F32, tag="x")
                for rr2 in range(RPC):
                    r_g = chk * RPC + rr2
                    nc.tensor.matmul(ps_wb[:, rr2 * T:(rr2 + 1) * T],
                                     selm_s[:, r_g * 128:(r_g + 1) * 128],
                                     WTe, start=True, stop=True)
                wrow_s = p_row.tile([1, TCH], F32, tag="wrow")
                nc.scalar.copy(wrow_s, ps_wb[0:1, :])
                xw = p_xw.tile([128, KT, TCH], _dt("w1"), tag="vxw")
                for kt in range(KT):
                    xg = p_gth.tile([128, RPC, T], F32, tag="xg")
                    nc.sync.dma_start(
                        xg, ag_out[chk * RPC:(chk + 1) * RPC,
                                   kt * 128:(kt + 1) * 128, :]
                        .rearrange("r p t -> p r t"))
                    nc.vector.tensor_tensor(
                        xw[:, kt], xg.rearrange("p r t -> p (r t)"),
                        ps_wb, ALU.mult)
                h_s = p_big.tile([128, MTH, TCH], _dt("w2"), tag="big")
                for mt in range(MTH):
                    w1t = p_wst.tile([128, KT, 128], _dt("w1"), tag="wtile")
                    nc.sync.dma_start(w1t, w1e_d.ap()[l, mt])
                    ps_h = ppMM.tile([128, TCH], F32, tag="mm")
                    for kt in range(KT):
                        nc.tensor.matmul(ps_h, w1t[:, kt],
                                         xw[:, kt],
                                         start=(kt == 0), stop=False)
                    nc.tensor.matmul(ps_h, b1_s[:, mt * 128:(mt + 1) * 128],
                                     wrow_s, start=False, stop=True)
                    nc.scalar.activation(out=h_s[:, mt], in_=ps_h, func=AF.Relu)
                for mt in range(KT):
                    w2t = p_wst.tile([128, MTH, 128], _dt("w2"), tag="w2t")
                    nc.sync.dma_start(w2t, w2e_d.ap()[l, mt])
                    ps_y = ppMM.tile([128, TCH], F32, tag="mm")
                    for kt in range(MTH):
                        nc.tensor.matmul(
                            ps_y, w2t[:, kt],
                            h_s[:, kt], start=(kt == 0), stop=False)
                    nc.tensor.matmul(ps_y, b2_s[:, mt * 128:(mt + 1) * 128],
                                     wrow_s, start=False, stop=True)
                    y_sb = p_attn.tile([128, TCH], F32, tag="ysb")
                    nc.vector.tensor_copy(y_sb, ps_y)
                    nc.sync.dma_start(
                        z_dram[chk * RPC:(chk + 1) * RPC, mt, :, :]
                        .rearrange("r p t -> p r t"), y_sb)
            nc.gpsimd.collective_compute(
                "ReduceScatter", ALU.add, replica_groups=rg,
                ins=[z_dram.opt()], outs=[rs_out.opt()])
            m_s = p_x.tile([128, KT, T], F32, tag="xbuf")
            nc.sync.dma_start(m_s, rs_out.rearrange("k p t -> p k t"))
            for kt in range(KT):
                nc.vector.tensor_add(u[:, kt], u[:, kt], m_s[:, kt])
            x_cur = layernorm(u, ln2g_d, ln2b_d, l)

        # ---------- pool + classifier ----------
        pool_c = p_sm.tile([128, KT], F32, tag="pool")
        for kt in range(KT):
            rsm = p_sm.tile([128, 1], F32, tag="psum1")
            nc.vector.reduce_sum(rsm, x_cur[:, kt], axis=mybir.AxisListType.X)
            nc.vector.tensor_scalar_mul(pool_c[:, kt:kt + 1], rsm, 1.0 / S)
        cls_s = p_sm.tile([128, KT, C], F32, tag="clsw")
        nc.sync.dma_start(cls_s, clsT_d.ap().rearrange("(k p) c -> p k c", p=128))
        ps_c = ppS.tile([C, 1], F32, tag="sm")
        for kt in range(KT):
            nc.tensor.matmul(ps_c, cls_s[:, kt], pool_c[:, kt:kt + 1],
                             start=(kt == 0), stop=False)
        nc.tensor.matmul(ps_c, clsb_s, one1, start=False, stop=True)
        cls_sb = p_sm.tile([C, 1], F32, tag="clso")
        nc.vector.tensor_copy(cls_sb, ps_c)
        nc.sync.dma_start(cls_out_d.ap(), cls_sb)

    nc.compile()
    return nc


_NC_CACHE = None


def kernel(**inputs):
    global _NC_CACHE
    inp = {k: np.asarray(v) for k, v in inputs.items()}
    pe = _pos_encoding_np()

    def _tiles(wT, nmt):
        # wT: [L, K, M] (lhsT layout) -> [L, nmt, 128, K//128, 128] contiguous tiles
        Lw, Kw, Mw = wT.shape
        return np.ascontiguousarray(
            wT.reshape(Lw, Kw // 128, 128, nmt, 128).transpose(0, 3, 2, 1, 4))

    ipw = inp["in_proj_w"]
    wqkT = _tiles(ipw[:, :2 * D, :].transpose(0, 2, 1), 16)
    wvT = np.ascontiguousarray(ipw[:, 2 * D:, :].transpose(0, 2, 1))
    ipb = inp["in_proj_b"]
    common = dict(
        emb=inp["emb"],
        wqkT=wqkT,
        bqk=ipb[:, :2 * D].reshape(L, 16, 128),
        wvT=wvT,
        bv=ipb[:, 2 * D:].reshape(L, 1, D),
        owT=_tiles(inp["out_w"].transpose(0, 2, 1), KT),
        ob=inp["out_b"].reshape(L, 1, D),
        ln1g=inp["ln1_g"].reshape(L, KT, 128),
        ln1b=inp["ln1_b"].reshape(L, KT, 128),
        ln2g=inp["ln2_g"].reshape(L, KT, 128),
        ln2b=inp["ln2_b"].reshape(L, KT, 128),
        gwT=inp["gate_w"].transpose(0, 2, 1),
        gb=inp["gate_b"].reshape(L, 1, E),
        clsT=inp["cls_w"].T,
        selm=np.kron(np.eye(8, dtype=np.float32), np.ones((1, 128), np.float32)),
        clsb=inp["cls_b"].reshape(1, C),
    )
    common = {k: np.ascontiguousarray(v, dtype=np.float32) for k, v in common.items()}

    in_maps = []
    for c in range(N_CORES):
        onehot = np.zeros((8, 1), np.float32)
        onehot[c, 0] = 1.0
        m = dict(common)
        m["srcl"] = np.ascontiguousarray(inp["src"][c].reshape(2, 128))
        m["pe"] = np.ascontiguousarray(pe[c].reshape(KT, 128))
        m["onehot"] = onehot
        m["w1e"] = _tiles(inp["w1"][:, c].astype(np.float32), MTH)
        m["b1e"] = np.ascontiguousarray(inp["b1"][:, c].reshape(L, 1, HFF),
                                        dtype=np.float32)
        m["w2e"] = _tiles(inp["w2"][:, c].astype(np.float32), KT)
        m["b2e"] = np.ascontiguousarray(inp["b2"][:, c].reshape(L, 1, D),
                                        dtype=np.float32)
        in_maps.append(m)

    if _NC_CACHE is None:
        _NC_CACHE = build_kernel()
    nc = _NC_CACHE

    res = run_bass_kernel_spmd(nc, in_maps, core_ids=list(range(N_CORES)),
                               trace=bool(os.environ.get("KTRACE")))
    if os.environ.get("KTRACE"):
        kernel.last_exec_ns = res.exec_time_ns
        kernel.last_trace = res.instructions_and_trace

    out = np.stack([res.results[c]["cls_out"][:, 0] for c in range(N_CORES)])
    gates = np.stack([res.results[c]["gates_out"].reshape(L, S, E)
                      for c in range(N_CORES)], axis=1)
    return out, gates


# revision 28
# speedup vs baseline: 1.1716x; 1.0621x over previous
import contextlib
import math
import os

import numpy as np

import concourse.bass as bass
import concourse.mybir as mybir
import concourse.tile as tile
from concourse import bacc
from concourse.bass_utils import run_bass_kernel_spmd
from concourse.masks import make_identity

# Model dims (hardcoded for nn_MoETransformerClassifier_76433237999751)
B, S, D, NH, HFF, E, TOPK, C, L, V = 8, 256, 1024, 16, 2048, 8, 2, 10, 4, 32000
HD = D // NH          # 64
T = S                 # 256 local tokens per core
KT = D // 128         # 8 k-tiles over D
MTH = HFF // 128      # 16 m-tiles over HFF
N_CORES = 8
EPS = 1e-5
TCH = 512             # token chunk for expert FFN
NCH = (N_CORES * T) // TCH  # 4 chunks of global tokens
RPC = TCH // T        # ranks per chunk = 2

F32 = mybir.dt.float32
F32R = mybir.dt.float32r
AF = mybir.ActivationFunctionType
ALU = mybir.AluOpType

# Per-matmul-group precision: "f32r" (full PE rate, rel err ~1.5e-4) or "f32"
# (exact, 4 cyc/row). Gate/stat/bias/select matmuls are always fp32.
# qkv/v read the fp32 residual stream directly -> fp32 (mixed dtypes illegal).
# av uses tile_position col-tiling, which fp32r's weight path cannot encode.
PREC = dict(qkv="f32r", v="f32r", score="f32r", av="f32", oproj="f32r",
            w1="f32r", w2="f32r")
_env = os.environ.get("KPREC")
if _env:
    for kv in _env.split(","):
        k, vv = kv.split("=")
        if k == "all":
            for kk in PREC:
                PREC[kk] = vv
        else:
            PREC[k] = vv


def _dt(group):
    return F32R if PREC[group] == "f32r" else F32


def _pos_encoding_np():
    pos = np.arange(5000, dtype=np.float32)[:, None]
    div = np.exp(np.arange(0, D, 2, dtype=np.float32) * (-math.log(10000.0) / D))
    pe = np.zeros((5000, D), dtype=np.float32)
    pe[:, 0::2] = np.sin(pos * div)
    pe[:, 1::2] = np.cos(pos * div)
    return pe[:B]  # [B, D]


def build_kernel(zb=()):
    nc = bacc.Bacc(None, target_bir_lowering=False, debug=False,
                   num_devices=N_CORES)
    zb = set(zb)  # names of bias inputs that are all-zero -> skip their matmuls

    def din(name, shape, dtype=F32):
        return nc.dram_tensor(name, shape, dtype, kind="ExternalInput")

    emb_d = din("emb", [V, D])
    srcl_d = din("srcl", [2, 128], mybir.dt.int32)
    pe_d = din("pe", [KT, 128])
    onehot_d = din("onehot", [8, 1])
    wqkT_d = din("wqkT", [L, 16, 128, KT, 128], _dt("qkv"))
    bqk_d = din("bqk", [L, 16, 128])
    wvT_d = din("wvT", [L, D, D], _dt("v"))
    bv_d = din("bv", [L, 1, D])
    owT_d = din("owT", [L, KT, 128, KT, 128], _dt("oproj"))
    ob_d = din("ob", [L, 1, D])
    ln1g_d = din("ln1g", [L, KT, 128])
    ln1b_d = din("ln1b", [L, KT, 128])
    ln2g_d = din("ln2g", [L, KT, 128])
    ln2b_d = din("ln2b", [L, KT, 128])
    gwT_d = din("gwT", [L, D, E])
    gb_d = din("gb", [L, 1, E])
    w1e_d = din("w1e", [L, MTH, 128, KT, 128], _dt("w1"))
    b1e_d = din("b1e", [L, 1, HFF])
    w2e_d = din("w2e", [L, KT, 128, MTH, 128], _dt("w2"))
    b2e_d = din("b2e", [L, 1, D])
    selm_d = din("selm", [8, 8 * 128])
    clsT_d = din("clsT", [D, C])
    clsb_d = din("clsb", [1, C])

    cls_out_d = nc.dram_tensor("cls_out", [C, 1], F32, kind="ExternalOutput")
    gates_out_d = nc.dram_tensor("gates_out", [L, 2, 128, E], F32,
                                 kind="ExternalOutput")

    rg = [list(range(N_CORES))]

    with tile.TileContext(nc) as tc, contextlib.ExitStack() as ctx:
        dram = ctx.enter_context(tc.tile_pool(name="dram", bufs=1, space="DRAM"))
        singles = ctx.enter_context(tc.tile_pool(name="singles", bufs=1))
        p_x = ctx.enter_context(tc.tile_pool(name="p_x", bufs=2))
        p_big = ctx.enter_context(tc.tile_pool(name="p_big", bufs=1))
        p_xw = ctx.enter_context(tc.tile_pool(name="p_xw", bufs=1))
        p_oT = ctx.enter_context(tc.tile_pool(name="p_oT", bufs=1))
        p_wst = ctx.enter_context(tc.tile_pool(name="p_wst", bufs=2))
        p_attn = ctx.enter_context(tc.tile_pool(name="p_attn", bufs=2))
        p_gth = ctx.enter_context(tc.tile_pool(name="p_gth", bufs=2))
        p_sm = ctx.enter_context(tc.tile_pool(name="p_sm", bufs=2))
        p_row = ctx.enter_context(tc.tile_pool(name="p_row", bufs=1))
        ppMM = ctx.enter_context(tc.tile_pool(name="ppMM", bufs=3, space="PSUM"))
        ppX = ctx.enter_context(tc.tile_pool(name="ppX", bufs=3, space="PSUM"))
        ppS = ctx.enter_context(tc.tile_pool(name="ppS", bufs=2, space="PSUM"))

        ident = singles.tile([128, 128], F32)
        make_identity(nc, ident[:])
        identr = singles.tile([128, 128], F32R)
        nc.vector.tensor_copy(identr, ident)
        ones_col = singles.tile([128, 1], F32)
        nc.vector.memset(ones_col, 1.0)
        ones_row = singles.tile([1, TCH], F32)
        nc.vector.memset(ones_row, 1.0)
        pe_s = singles.tile([128, KT], F32)
        nc.sync.dma_start(pe_s, pe_d.ap().rearrange("k p -> p k"))
        oh_s = singles.tile([8, 1], F32)
        nc.sync.dma_start(oh_s, onehot_d.ap())
        clsb_s = singles.tile([1, C], F32)
        nc.sync.dma_start(clsb_s, clsb_d.ap())
        one1 = singles.tile([1, 1], F32)
        nc.vector.memset(one1, 1.0)
        selm_s = singles.tile([8, 8 * 128], F32)
        nc.sync.dma_start(selm_s, selm_d.ap())
        eps_s = singles.tile([1, 1], F32)
        nc.vector.memset(eps_s, EPS)

        ag_in = dram.tile([KT * 128 + 8, T], F32)
        ag_outs = [dram.tile([N_CORES, KT * 128 + 8, T], F32,
                             addr_space="Shared", tag=f"ag{i}", name=f"ag_out{i}")
                   for i in range(L)]
        z_dram = dram.tile([N_CORES, KT, 128, T], F32)
        rs_out = dram.tile([KT, 128, T], F32)

        # ---------- embedding gather -> xT (feature-major) ----------
        xT = p_x.tile([128, KT, T], F32, tag="xbuf")
        for h in range(2):
            idx = p_sm.tile([128, 1], mybir.dt.int32, tag="idx")
            nc.sync.dma_start(idx, srcl_d.ap()[h].rearrange("(t o) -> t o", o=1))
            gt = p_xw.tile([128, D], F32, tag="vxw")
            nc.gpsimd.indirect_dma_start(
                out=gt[:], out_offset=None, in_=emb_d.ap(),
                in_offset=bass.IndirectOffsetOnAxis(ap=idx[:, :1], axis=0))
            for kt in range(KT):
                pt = ppX.tile([128, TCH], F32, tag="x")
                nc.tensor.transpose(pt[:, :128], gt[:, kt * 128:(kt + 1) * 128],
                                    ident[:])
                nc.scalar.activation(
                    out=xT[:, kt, h * 128:(h + 1) * 128], in_=pt[:, :128],
                    func=AF.Identity, scale=32.0, bias=pe_s[:, kt:kt + 1])

        def layernorm(xin, g_d, b_d, l):
            ps_sum = ppS.tile([1, T], F32, tag="sm")
            ps_sq = ppS.tile([1, T], F32, tag="sm")
            for kt in range(KT):
                sq_t = p_attn.tile([128, T], F32, tag="sqt")
                nc.scalar.activation(out=sq_t, in_=xin[:, kt], func=AF.Square)
                nc.tensor.matmul(ps_sum, ones_col[:], xin[:, kt],
                                 start=(kt == 0), stop=(kt == KT - 1))
                nc.tensor.matmul(ps_sq, ones_col[:], sq_t,
                                 start=(kt == 0), stop=(kt == KT - 1))
            mu = p_row.tile([1, T], F32, tag="rA")
            nc.scalar.activation(out=mu, in_=ps_sum, func=AF.Copy, scale=1.0 / D)
            ps_mu = ppX.tile([128, TCH], F32, tag="x")
            nc.tensor.matmul(ps_mu[:, :T], ones_row[:1, :128], mu,
                             start=True, stop=True)
            msq = p_row.tile([1, T], F32, tag="rB")
            nc.scalar.activation(out=msq, in_=ps_sq, func=AF.Copy, scale=1.0 / D)
            mu2 = p_row.tile([1, T], F32, tag="rC")
            nc.vector.tensor_mul(mu2, mu, mu)
            var = p_row.tile([1, T], F32, tag="rA")
            nc.vector.tensor_tensor(var, msq, mu2, ALU.subtract)
            vpe = p_row.tile([1, T], F32, tag="rB")
            nc.vector.tensor_scalar_add(vpe, var, EPS)
            lnv = p_row.tile([1, T], F32, tag="rC")
            nc.scalar.activation(out=lnv, in_=var, func=AF.Ln, bias=eps_s[:1])
            r0 = p_row.tile([1, T], F32, tag="rA")
            nc.scalar.activation(out=r0, in_=lnv, func=AF.Exp, scale=-0.5)
            # one Newton step: r' = r0*(1.5 - 0.5*(var+eps)*r0^2)
            rr = p_row.tile([1, T], F32, tag="rC")
            nc.vector.tensor_mul(rr, r0, r0)
            hv = p_row.tile([1, T], F32, tag="rD")
            nc.vector.tensor_scalar_mul(hv, vpe, -0.5)
            a1 = p_row.tile([1, T], F32, tag="rB")
            nc.vector.tensor_mul(a1, hv, rr)
            nc.vector.tensor_scalar_add(a1, a1, 1.5)
            rstd = p_row.tile([1, T], F32, tag="rC")
            nc.vector.tensor_mul(rstd, r0, a1)
            ps_rs = ppX.tile([128, TCH], F32, tag="x")
            nc.tensor.matmul(ps_rs[:, :T], ones_row[:1, :128], rstd,
                             start=True, stop=True)
            g_s = p_sm.tile([128, KT], F32, tag="lng")
            nc.sync.dma_start(g_s, g_d.ap()[l].rearrange("k p -> p k"))
            b_s = p_sm.tile([128, KT], F32, tag="lnb")
            nc.sync.dma_start(b_s, b_d.ap()[l].rearrange("k p -> p k"))
            xout = p_x.tile([128, KT, T], F32, tag="xbuf")
            for kt in range(KT):
                tt = p_attn.tile([128, T], F32, tag="lntmp")
                nc.vector.tensor_tensor(tt, xin[:, kt], ps_mu[:, :T], ALU.subtract)
                nc.vector.tensor_tensor(tt, tt, ps_rs[:, :T], ALU.mult)
                nc.scalar.activation(out=xout[:, kt], in_=tt, func=AF.Identity,
                                     scale=g_s[:, kt:kt + 1],
                                     bias=b_s[:, kt:kt + 1])
            return xout

        x_cur = xT
        for l in range(L):
            # fp32r view of the stream for attention input matmuls (the fp32
            # residual stream itself stays exact for routing)
            if PREC["qkv"] == "f32r":
                xr = p_x.tile([128, KT, T], F32R, tag="xr", bufs=1)
                for kt in range(KT):
                    nc.gpsimd.tensor_copy(out=xr[:, kt], in_=x_cur[:, kt])
            else:
                xr = x_cur
            # ---------- attention: QK (feature-major) ----------
            qk_s = p_big.tile([128, 16, T], _dt("score"), tag="big")
            bqk_s = p_sm.tile([128, 16], F32, tag="bqk")
            nc.sync.dma_start(bqk_s, bqk_d.ap()[l].rearrange("m p -> p m"))
            for mt in range(16):
                wt = p_wst.tile([128, KT, 128], _dt("qkv"), tag="wtile")
                nc.sync.dma_start(wt, wqkT_d.ap()[l, mt])
                ps = ppMM.tile([128, TCH], F32, tag="mm")
                for kt in range(KT):
                    nc.tensor.matmul(ps[:, :T], wt[:, kt],
                                     xr[:, kt],
                                     start=(kt == 0), stop=(kt == KT - 1))
                nc.scalar.activation(out=qk_s[:, mt], in_=ps[:, :T],
                                     func=AF.Identity,
                                     bias=bqk_s[:, mt:mt + 1])
            # ---------- V (token-major) ----------
            v_s = p_xw.tile([128, KT, TCH], _dt("av"), tag="vxw")
            bv_s = p_row.tile([1, D], F32, tag="biasrow")
            nc.sync.dma_start(bv_s, bv_d.ap()[l])
            for mt_tok in range(2):
                ps0 = ppMM.tile([128, TCH], F32, tag="mm")
                ps1 = ppMM.tile([128, TCH], F32, tag="mm")
                pss = [ps0, ps1]
                for kt in range(KT):
                    vw = p_wst.tile([128, D], _dt("v"), tag="vw")
                    nc.sync.dma_start(vw, wvT_d.ap()[l][kt * 128:(kt + 1) * 128, :])
                    for nh2 in range(2):
                        nc.tensor.matmul(
                            pss[nh2],
                            xr[:, kt, mt_tok * 128:(mt_tok + 1) * 128],
                            vw[:, nh2 * 512:(nh2 + 1) * 512],
                            start=(kt == 0),
                            stop=(kt == KT - 1 and "bv" in zb))
                for nh2 in range(2):
                    if "bv" not in zb:
                        nc.tensor.matmul(pss[nh2], ones_row[:1, :128],
                                         bv_s[:, nh2 * 512:(nh2 + 1) * 512],
                                         start=False, stop=True)
                    nc.scalar.copy(v_s[:, mt_tok * 2 + nh2, :], pss[nh2])
            # v layout: v_s[:, mt_tok*2 + nh2, c] = V[token tile mt_tok, feature nh2*512+c]
            # ---------- attention heads ----------
            oT_s = p_oT.tile([128, 8, T], _dt("oproj"), tag="oT")
            for h in range(NH):
                mt_q = h // 2
                r0 = (h % 2) * 64
                attnT = p_attn.tile([128, 2, T], _dt("av"), tag="attnT")
                for qh in range(2):
                    ps_sc = ppMM.tile([128, TCH], F32, tag="mm")
                    nc.tensor.matmul(
                        ps_sc[:, :T],
                        qk_s[r0:r0 + 64, mt_q, qh * 128:(qh + 1) * 128],
                        qk_s[r0:r0 + 64, 8 + mt_q, :],
                        start=True, stop=True)
                    mx = p_sm.tile([128, 1], F32, tag="mx")
                    nc.vector.reduce_max(mx, ps_sc[:, :T], axis=mybir.AxisListType.X)
                    nmx = p_sm.tile([128, 1], F32, tag="nmx")
                    nc.vector.tensor_scalar_mul(nmx, mx, -0.125)
                    esum = p_sm.tile([128, 1], F32, tag="esum")
                    ae = p_attn.tile([128, T], F32, tag="ae")
                    nc.scalar.activation(out=ae, in_=ps_sc[:, :T], func=AF.Exp,
                                         scale=0.125, bias=nmx, accum_out=esum)
                    rs = p_sm.tile([128, 1], F32, tag="rsum")
                    nc.vector.reciprocal(rs, esum)
                    an = p_attn.tile([128, T], _dt("av"), tag="an")
                    nc.vector.tensor_scalar_mul(an, ae, rs)
                    for kh in range(2):
                        pt = ppX.tile([128, TCH], _dt("av"), tag="x")
                        nc.tensor.transpose(
                            pt[:, :128], an[:, kh * 128:(kh + 1) * 128],
                            identr[:] if _dt("av") == F32R else ident[:])
                        nc.scalar.copy(attnT[:, kh, qh * 128:(qh + 1) * 128], pt[:, :128])
                if h % 2 == 0:
                    ps_o = ppMM.tile([128, TCH], F32, tag="mm")
                for kt2 in range(2):
                    # v slice for head h, token-tile kt2:
                    # d-range h*64:(h+1)*64 lives in nh2 = h//8, col h%8*64...
                    nh2 = (h * 64) // 512
                    c0 = (h * 64) % 512
                    vsl = v_s[:, kt2 * 2 + nh2, c0:c0 + 64]
                    nc.tensor.matmul(
                        ps_o[r0:r0 + 64, :T], vsl,
                        attnT[:, kt2, :],
                        start=(kt2 == 0), stop=(kt2 == 1),
                        tile_position=(0, r0))
                if h % 2 == 1:
                    nc.scalar.copy(oT_s[:, h // 2, :], ps_o[:, :T])
            # ---------- out-proj + residual ----------
            x2 = p_x.tile([128, KT, T], F32, tag="xbuf")
            ob_s = p_row.tile([1, D], F32, tag="biasrow")
            nc.sync.dma_start(ob_s, ob_d.ap()[l])
            for mt in range(KT):
                wt = p_wst.tile([128, KT, 128], _dt("oproj"), tag="wtile")
                nc.sync.dma_start(wt, owT_d.ap()[l, mt])
                ps = ppMM.tile([128, TCH], F32, tag="mm")
                for kt in range(KT):
                    nc.tensor.matmul(ps[:, :T], wt[:, kt],
                                     oT_s[:, kt],
                                     start=(kt == 0),
                                     stop=(kt == KT - 1 and "ob" in zb))
                if "ob" not in zb:
                    nc.tensor.matmul(ps[:, :T],
                                     ob_s[:, mt * 128:(mt + 1) * 128],
                                     ones_row[:1, :T], start=False, stop=True)
                nc.vector.scalar_tensor_tensor(
                    out=x2[:, mt], in0=ps[:, :T], scalar=1.0, in1=x_cur[:, mt],
                    op0=ALU.mult, op1=ALU.add)
            u = layernorm(x2, ln1g_d, ln1b_d, l)
            # ---------- gate + top-2 ----------
            gw_s = p_sm.tile([128, KT, E], F32, tag="gw")
            nc.sync.dma_start(gw_s,
                              gwT_d.ap()[l].rearrange("(k p) e -> p k e", p=128))
            gb_s = p_row.tile([1, E], F32, tag="gb")
            nc.sync.dma_start(gb_s, gb_d.ap()[l])
            W_s = p_sm.tile([128, 2, E], F32, tag="Ws")
            for mt_tok in range(2):
                ps_g = ppS.tile([128, E], F32, tag="sm")
                for kt in range(KT):
                    nc.tensor.matmul(
                        ps_g, u[:, kt, mt_tok * 128:(mt_tok + 1) * 128],
                        gw_s[:, kt], start=(kt == 0),
                        stop=(kt == KT - 1 and "gb" in zb))
                if "gb" not in zb:
                    nc.tensor.matmul(ps_g, ones_row[:1, :128], gb_s,
                                     start=False, stop=True)
                logit = p_sm.tile([128, E], F32, tag="logit")
                nc.vector.tensor_copy(logit, ps_g)
                nc.sync.dma_start(gates_out_d.ap()[l, mt_tok], logit)
                m1 = p_sm.tile([128, 1], F32, tag="m1")
                nc.vector.reduce_max(m1, logit, axis=mybir.AxisListType.X)
                eq1 = p_sm.tile([128, E], F32, tag="eq1")
                nc.vector.tensor_scalar(out=eq1, in0=logit, scalar1=m1,
                                        scalar2=None, op0=ALU.is_equal)
                msk = p_sm.tile([128, E], F32, tag="msk")
                nc.vector.scalar_tensor_tensor(
                    out=msk, in0=eq1, scalar=-1e30, in1=logit,
                    op0=ALU.mult, op1=ALU.add)
                m2 = p_sm.tile([128, 1], F32, tag="m2")
                nc.vector.reduce_max(m2, msk, axis=mybir.AxisListType.X)
                eq2 = p_sm.tile([128, E], F32, tag="eq2")
                nc.vector.tensor_scalar(out=eq2, in0=msk, scalar1=m2,
                                        scalar2=None, op0=ALU.is_equal)
                nm1 = p_sm.tile([128, 1], F32, tag="nm1")
                nc.vector.tensor_scalar_mul(nm1, m1, -1.0)
                e2 = p_sm.tile([128, 1], F32, tag="e2")
                nc.scalar.activation(out=e2, in_=m2, func=AF.Exp, bias=nm1)
                den = p_sm.tile([128, 1], F32, tag="den")
                nc.vector.tensor_scalar_add(den, e2, 1.0)
                inv = p_sm.tile([128, 1], F32, tag="inv")
                nc.vector.reciprocal(inv, den)
                w2c = p_sm.tile([128, 1], F32, tag="w2c")
                nc.vector.tensor_mul(w2c, e2, inv)
                t1 = p_sm.tile([128, E], F32, tag="t1")
                nc.vector.tensor_scalar_mul(t1, eq1, inv)
                t2 = p_sm.tile([128, E], F32, tag="t2")
                nc.vector.tensor_scalar_mul(t2, eq2, w2c)
                nc.vector.tensor_add(W_s[:, mt_tok], t1, t2)
            # ---------- exchange: AllGather (u, W^T) ----------
            WT_sb = p_row.tile([8, T], F32, tag="wtsb")
            for mt_tok in range(2):
                pt = ppX.tile([128, TCH], F32, tag="x")
                nc.tensor.transpose(pt[:8, :128], W_s[:, mt_tok], ident[:])
                nc.scalar.copy(WT_sb[:, mt_tok * 128:(mt_tok + 1) * 128],
                               pt[:8, :128])
            nc.sync.dma_start(
                ag_in[:KT * 128, :].rearrange("(k p) t -> p k t", p=128), u)
            nc.sync.dma_start(ag_in[KT * 128:, :], WT_sb)
            ag_out = ag_outs[l]
            nc.gpsimd.collective_compute(
                "AllGather", ALU.bypass, replica_groups=rg,
                ins=[ag_in.opt()], outs=[ag_out.opt()])
            WTe = p_row.tile([8, T], F32, tag="wte")
            for r in range(N_CORES):
                wrows = p_sm.tile([8, T], F32, tag="wrows")
                nc.sync.dma_start(wrows, ag_out[r, KT * 128:, :])
                ps_sel = ppS.tile([1, T], F32, tag="sm")
                nc.tensor.matmul(ps_sel, oh_s[:, :1], wrows, start=True, stop=True)
                selrow = p_sm.tile([1, T], F32, tag="selrow")
                nc.scalar.copy(selrow, ps_sel)
                nc.sync.dma_start(WTe[r:r + 1, :], selrow)
            # ---------- expert FFN (this core's expert) ----------
            b1_s = p_row.tile([1, HFF], F32, tag="biasrow")
            nc.sync.dma_start(b1_s, b1e_d.ap()[l])
            b2_s = p_row.tile([1, D], F32, tag="biasrow2")
            nc.sync.dma_start(b2_s, b2e_d.ap()[l])
            for chk in range(NCH):
                ps_wb = ppX.tile([128, TCH], F32, tag="x")
                for rr2 in range(RPC):
                    r_g = chk * RPC + rr2
                    nc.tensor.matmul(ps_wb[:, rr2 * T:(rr2 + 1) * T],
                                     selm_s[:, r_g * 128:(r_g + 1) * 128],
                                     WTe, start=True, stop=True)
                wrow_s = p_row.tile([1, TCH], F32, tag="wrow")
                nc.scalar.copy(wrow_s, ps_wb[0:1, :])
                xw = p_xw.tile([128, KT, TCH], _dt("w1"), tag="vxw")
                for kt in range(KT):
                    xg = p_gth.tile([128, RPC, T], F32, tag="xg")
                    nc.sync.dma_start(
                        xg, ag_out[chk * RPC:(chk + 1) * RPC,
                                   kt * 128:(kt + 1) * 128, :]
                        .rearrange("r p t -> p r t"))
                    nc.vector.tensor_tensor(
                        xw[:, kt], xg.rearrange("p r t -> p (r t)"),
                        ps_wb, ALU.mult)
                h_s = p_big.tile([128, MTH, TCH], _dt("w2"), tag="big")
                for mt in range(MTH):
                    w1t = p_wst.tile([128, KT, 128], _dt("w1"), tag="wtile")
                    nc.sync.dma_start(w1t, w1e_d.ap()[l, mt])
                    ps_h = ppMM.tile([128, TCH], F32, tag="mm")
                    for kt in range(KT):
                        nc.tensor.matmul(ps_h, w1t[:, kt],
                                         xw[:, kt],
                                         start=(kt == 0),
                                         stop=(kt == KT - 1 and "b1e" in zb))
                    if "b1e" not in zb:
                        nc.tensor.matmul(ps_h, b1_s[:, mt * 128:(mt + 1) * 128],
                                         wrow_s, start=False, stop=True)
                    nc.scalar.activation(out=h_s[:, mt], in_=ps_h, func=AF.Relu)
                for mt in range(KT):
                    w2t = p_wst.tile([128, MTH, 128], _dt("w2"), tag="w2t")
                    nc.sync.dma_start(w2t, w2e_d.ap()[l, mt])
                    ps_y = ppMM.tile([128, TCH], F32, tag="mm")
                    for kt in range(MTH):
                        nc.tensor.matmul(
                            ps_y, w2t[:, kt],
                            h_s[:, kt], start=(kt == 0),
                            stop=(kt == MTH - 1 and "b2e" in zb))
                    if "b2e" not in zb:
                        nc.tensor.matmul(ps_y, b2_s[:, mt * 128:(mt + 1) * 128],
                                         wrow_s, start=False, stop=True)
                    y_sb = p_attn.tile([128, TCH], F32, tag="ysb")
                    nc.vector.tensor_copy(y_sb, ps_y)
                    nc.sync.dma_start(
                        z_dram[chk * RPC:(chk + 1) * RPC, mt, :, :]
                        .rearrange("r p t -> p r t"), y_sb)
            nc.gpsimd.collective_compute(
                "ReduceScatter", ALU.add, replica_groups=rg,
                ins=[z_dram.opt()], outs=[rs_out.opt()])
            m_s = p_x.tile([128, KT, T], F32, tag="xbuf")
            nc.sync.dma_start(m_s, rs_out.rearrange("k p t -> p k t"))
            for kt in range(KT):
                nc.vector.tensor_add(u[:, kt], u[:, kt], m_s[:, kt])
            x_cur = layernorm(u, ln2g_d, ln2b_d, l)

        # ---------- pool + classifier ----------
        pool_c = p_sm.tile([128, KT], F32, tag="pool")
        for kt in range(KT):
            rsm = p_sm.tile([128, 1], F32, tag="psum1")
            nc.vector.reduce_sum(rsm, x_cur[:, kt], axis=mybir.AxisListType.X)
            nc.vector.tensor_scalar_mul(pool_c[:, kt:kt + 1], rsm, 1.0 / S)
        cls_s = p_sm.tile([128, KT, C], F32, tag="clsw")
        nc.sync.dma_start(cls_s, clsT_d.ap().rearrange("(k p) c -> p k c", p=128))
        ps_c = ppS.tile([C, 1], F32, tag="sm")
        for kt in range(KT):
            nc.tensor.matmul(ps_c, cls_s[:, kt], pool_c[:, kt:kt + 1],
                             start=(kt == 0),
                             stop=(kt == KT - 1 and "clsb" in zb))
        if "clsb" not in zb:
            nc.tensor.matmul(ps_c, clsb_s, one1, start=False, stop=True)
        cls_sb = p_sm.tile([C, 1], F32, tag="clso")
        nc.vector.tensor_copy(cls_sb, ps_c)
        nc.sync.dma_start(cls_out_d.ap(), cls_sb)

    nc.compile()
    return nc


_NC_CACHE = None


def kernel(**inputs):
    global _NC_CACHE
    inp = {k: np.asarray(v) for k, v in inputs.items()}
    pe = _pos_encoding_np()

    def _tiles(wT, nmt):
        # wT: [L, K, M] (lhsT layout) -> [L, nmt, 128, K//128, 128] contiguous tiles
        Lw, Kw, Mw = wT.shape
        return np.ascontiguousarray(
            wT.reshape(Lw, Kw // 128, 128, nmt, 128).transpose(0, 3, 2, 1, 4))

    ipw = inp["in_proj_w"]
    wqkT = _tiles(ipw[:, :2 * D, :].transpose(0, 2, 1), 16)
    wvT = np.ascontiguousarray(ipw[:, 2 * D:, :].transpose(0, 2, 1))
    ipb = inp["in_proj_b"]
    common = dict(
        emb=inp["emb"],
        wqkT=wqkT,
        bqk=ipb[:, :2 * D].reshape(L, 16, 128),
        wvT=wvT,
        bv=ipb[:, 2 * D:].reshape(L, 1, D),
        owT=_tiles(inp["out_w"].transpose(0, 2, 1), KT),
        ob=inp["out_b"].reshape(L, 1, D),
        ln1g=inp["ln1_g"].reshape(L, KT, 128),
        ln1b=inp["ln1_b"].reshape(L, KT, 128),
        ln2g=inp["ln2_g"].reshape(L, KT, 128),
        ln2b=inp["ln2_b"].reshape(L, KT, 128),
        gwT=inp["gate_w"].transpose(0, 2, 1),
        gb=inp["gate_b"].reshape(L, 1, E),
        clsT=inp["cls_w"].T,
        selm=np.kron(np.eye(8, dtype=np.float32), np.ones((1, 128), np.float32)),
        clsb=inp["cls_b"].reshape(1, C),
    )
    common = {k: np.ascontiguousarray(v, dtype=np.float32) for k, v in common.items()}

    in_maps = []
    for c in range(N_CORES):
        onehot = np.zeros((8, 1), np.float32)
        onehot[c, 0] = 1.0
        m = dict(common)
        m["srcl"] = np.ascontiguousarray(inp["src"][c].reshape(2, 128))
        m["pe"] = np.ascontiguousarray(pe[c].reshape(KT, 128))
        m["onehot"] = onehot
        m["w1e"] = _tiles(inp["w1"][:, c].astype(np.float32), MTH)
        m["b1e"] = np.ascontiguousarray(inp["b1"][:, c].reshape(L, 1, HFF),
                                        dtype=np.float32)
        m["w2e"] = _tiles(inp["w2"][:, c].astype(np.float32), KT)
        m["b2e"] = np.ascontiguousarray(inp["b2"][:, c].reshape(L, 1, D),
                                        dtype=np.float32)
        in_maps.append(m)

    zb = tuple(sorted(
        name for name, arr in [
            ("bv", common["bv"]), ("ob", common["ob"]), ("gb", common["gb"]),
            ("clsb", common["clsb"]),
            ("b1e", inp["b1"]), ("b2e", inp["b2"]),
        ] if not np.any(arr)))
    if _NC_CACHE is None or _NC_CACHE[0] != zb:
        _NC_CACHE = (zb, build_kernel(zb))
    nc = _NC_CACHE[1]

    res = run_bass_kernel_spmd(nc, in_maps, core_ids=list(range(N_CORES)),
                               trace=bool(os.environ.get("KTRACE")))
    if os.environ.get("KTRACE"):
        kernel.last_exec_ns = res.exec_time_ns
        kernel.last_trace = res.instructions_and_trace

    out = np.stack([res.results[c]["cls_out"][:, 0] for c in range(N_CORES)])
    gates = np.stack([res.results[c]["gates_out"].reshape(L, S, E)
                      for c in range(N_CORES)], axis=1)
    return out, gates
